# revision 20
# baseline (speedup 1.0000x reference)
"""Trainium2 Bass kernel for nn_MOA_13254269075617 (sparse windowed attention block).

Sharding: data-parallel over batch B=8 across 8 NeuronCores (1 image each).
BatchNorm uses global batch stats via one on-device AllReduce of per-channel
sum / sum-of-squares (plus an early warm-up collective to absorb CC-stream
startup cost).

Per-core pipeline (all in the spatially-TRANSPOSED frame T(z)[u,v]=z[v,u],
fp16 end-to-end so DVE elementwise ops run in 2x/4x perf modes):
  xT_cm  : x in channel-major [256, 4096], host-pre-permuted grid, loaded
           via 4 big transpose-DMAs
  vT     : (xT @ Wv + bv) token-major [128, 32, 256] fp16
  AE     : exp-logits pq-major [81, 4096] on a zero-padded 66-pitch grid
  W      : 25-tap position-varying stencil weights (fold+attention combined)
  vd     : 9 token-shifted copies of vT via SBUF->SBUF DMA (no PE work)
  acc    : 25-tap stencil apply, token-major fp16 FMAs on DVE (2x mode)
  x1/x2  : relu chains with 3x3/5x5 maxpools (separable shifted-max trees)
  out    : concat-matmul (Wfu) + residual, BN with AllReduce'd stats
  y      : channel-major fp16 output, host casts/transposes to f32 [H,W,C]
"""
import sys

for _p in (
    "/root/.axon_site",
    "/root/.axon_site/_ro/trn_rl_repo",
    "/root/.axon_site/_ro/pypackages",
    "/opt/trn_rl_repo",
):
    if _p not in sys.path:
        sys.path.append(_p)

from itertools import product

import numpy as np

import concourse.bass as bass
import concourse.tile as tile
from concourse import bacc, mybir
from concourse.bass_utils import run_bass_kernel_spmd

F32 = mybir.dt.float32
F16 = mybir.dt.float16
ALU = mybir.AluOpType
ACT = mybir.ActivationFunctionType

B, H, W, C = 8, 64, 64, 256
L = H * W                      # 4096 tokens
NCHUNK = L // 128              # 32 token chunks
N_CORES = 8
EPS = 1e-5

# (e, f) tap -> t index and source shift delta = 64*e + f
TAPI = {(e, f): (e + 2) * 5 + (f + 2) for e in range(-2, 3) for f in range(-2, 3)}


def host_consts():
    """Selector matrices and small constants (host-precomputed, same all cores)."""
    selsum = np.zeros((81, 9), np.float32)
    for p in range(9):
        selsum[9 * p:9 * p + 9, p] = 1.0
    selrep = np.zeros((9, 81), np.float32)
    for p in range(9):
        selrep[p, 9 * p:9 * p + 9] = 1.0
    # selshift[:, 25*d + tap]: for (di,dj) block d, tap (e,f):
    #   k = 9*(3di+dj) + 3(di+e)+(dj+f) if di+e,dj+f in [0,3)
    selshift = np.zeros((81, 9 * 25), np.float32)
    for d, (di, dj) in enumerate(product(range(3), range(3))):
        for t, (e, f) in enumerate(product(range(-2, 3), range(-2, 3))):
            dip, djp = di + e, dj + f
            if 0 <= dip < 3 and 0 <= djp < 3:
                k = 9 * (3 * di + dj) + (3 * dip + djp)
                selshift[k, 25 * d + t] = 1.0
    wmask = np.ones((25, 64, 64), np.float32)
    for t, (e, f) in enumerate(product(range(-2, 3), range(-2, 3))):
        if e > 0: wmask[t, 64 - e:, :] = 0
        if e < 0: wmask[t, :-e, :] = 0
        if f > 0: wmask[t, :, 64 - f:] = 0
        if f < 0: wmask[t, :, :-f] = 0
    f16 = np.float16
    return {
        "selsum": selsum.astype(f16),
        "selrep": selrep.astype(f16),
        "selshift": selshift.astype(f16),
        "wmask": wmask.reshape(25, 4096).astype(f16),
        "ident": np.eye(128, dtype=f16),
        "onesr": np.ones((1, 512), f16),
    }


def build(nc, n_cores, debug=False):
    d = {}
    def din(name, shape, dt=F16):
        d[name] = nc.dram_tensor(name, list(shape), dt, kind="ExternalInput").ap()
    def dout(name, shape, dt=F16):
        d[name] = nc.dram_tensor(name, list(shape), dt, kind="ExternalOutput").ap()

    din("xf16", (L, C))
    din("wv", (128, 2 * C)); din("bvrow", (1, C))
    din("wa", (128, 2 * 81)); din("barow", (81, 1), F32)
    din("wfu", (128, 4 * 2 * 128)); din("bfurow", (1, C))
    din("gamma2", (128, 2), F32); din("beta2", (128, 2), F32)
    din("selsum", (81, 9)); din("selrep", (9, 81)); din("selshift", (81, 225))
    din("ident", (128, 128)); din("onesr", (1, 512)); din("wmask", (25, L))
    dout("y", (2 * 128, L))
    if debug:
        dout("dbg_vt", (128, NCHUNK * C))
        dout("dbg_w", (25, L))
        dout("dbg_acc", (128, NCHUNK * C))
        dout("dbg_x1", (2 * 128, L))

    with tile.TileContext(nc) as tc:
        _build_tc(tc, d, n_cores, debug)
    return d


def _build_tc(tc, d, n_cores, debug):
    nc = tc.nc
    from contextlib import ExitStack
    es = ExitStack()
    with es:
        consts = es.enter_context(tc.tile_pool(name="consts", bufs=1))
        main = es.enter_context(tc.tile_pool(name="main", bufs=1))
        small = es.enter_context(tc.tile_pool(name="small", bufs=1))
        dram = es.enter_context(tc.tile_pool(name="dram", bufs=2, space="DRAM"))

        # ---- phase A: input loads first (run under the CC entry barrier) ----
        xT_cm = [main.tile([128, L], F16, tag=f"xcm{cc}", name=f"xT_cm{cc}")
                 for cc in range(2)]
        for cc in range(2):
            for q in range(2):
                nc.sync.dma_start_transpose(
                    xT_cm[cc][:, 2048 * q:2048 * (q + 1)],
                    d["xf16"][2048 * q:2048 * (q + 1), 128 * cc:128 * (cc + 1)])

        # ---- const loads on the scalar HWDGE queue ----
        def cload(name, shape, dt=F16, src=None):
            t = consts.tile(list(shape), dt, tag=name, name=name)
            nc.scalar.dma_start(t[:], (src if src is not None else d[name])[:])
            return t
        ident = cload("ident", (128, 128))
        onesr = cload("onesr", (1, 512))
        bv_row = cload("bvrow", (1, C))
        bfu_row = cload("bfurow", (1, C))
        ba_sb = cload("barow", (81, 1), F32)
        selsum = cload("selsum", (81, 9))
        selrep = cload("selrep", (9, 81))
        selshift = cload("selshift", (81, 225))
        gamma2 = cload("gamma2", (128, 2), F32)
        beta2 = cload("beta2", (128, 2), F32)
        wv_sb = cload("wv", (128, 2, C), src=d["wv"].rearrange("p (k c) -> p k c", k=2))
        wa_sb = cload("wa", (128, 2, 81), src=d["wa"].rearrange("p (k c) -> p k c", k=2))
        wfu_sb = cload("wfu", (128, 4, 2, 128),
                       src=d["wfu"].rearrange("p (k m c) -> p k m c", k=4, m=2))

        # ---- warm-up collective: absorbs CC stream startup (~20us) early ----
        warm_in = dram.tile([1, 1], F32, name="warm_in")
        warm_out = dram.tile([1, 1], F32, name="warm_out")
        nc.sync.dma_start(warm_in[:], ba_sb[0:1, 0:1])
        nc.gpsimd.collective_compute(
            "AllReduce", ALU.add, replica_groups=[list(range(n_cores))],
            ins=[warm_in.opt()], outs=[warm_out.opt()])

        # ---- phase C: attention logits -> exp -> normalize ----
        # AE grid: (g1=u, g2=v); AE[g1+1, g2+1] = softmax-numerator of the
        # ORIGINAL position (h=g2, w=g1) (x transposed-grid ordering).
        cmMid = tc.tile_pool(name="mid", bufs=1); mid = cmMid.__enter__()
        wmask = mid.tile([25, L], F16, tag="wmask", name="wmask")
        nc.scalar.dma_start(wmask[:], d["wmask"][:])
        cmCl = tc.tile_pool(name="psClog", bufs=1, space="PSUM"); psCl = cmCl.__enter__()
        AE = mid.tile([81, 66 * 67], F16, tag="AE", name="AE")
        AE3 = AE.rearrange("p (r s) -> p r s", r=67)
        nc.vector.memset(AE3[:, 0:1, :], 0.0)          # pad ring only; the
        nc.vector.memset(AE3[:, 65:67, :], 0.0)        # 64x64 interior is
        nc.vector.memset(AE3[:, 1:65, 0:1], 0.0)       # fully written by exp
        nc.vector.memset(AE3[:, 1:65, 65:66], 0.0)

        for grp in range(2):
            pss = [psCl.tile([81, 512], F32, tag=f"aps{q}", name=f"aps{q}")
                   for q in range(4)]
            for kc in range(2):
                for q in range(4):
                    n8 = 4 * grp + q
                    nc.tensor.matmul(pss[q][:], wa_sb[:, kc, :],
                                     xT_cm[kc][:, 512 * n8:512 * (n8 + 1)],
                                     start=(kc == 0), stop=(kc == 1))
            for q in range(4):
                n8 = 4 * grp + q
                nc.scalar.activation(AE3[:, 1 + 8 * n8:1 + 8 * n8 + 8, 1:65],
                                     pss[q].rearrange("p (r s) -> p r s", s=64),
                                     ACT.Exp, bias=ba_sb[:, 0:1])

        # ---- phase B: vT = xT @ Wv + bv, token-major fp16 ----
        # (emitted between C-logits and C-rowsums so vT lands early for the
        # shifted-copy DMAs; PE queue order = emission order)
        vT = main.tile([128, NCHUNK, C], F16, tag="vT", name="vT")
        cmB = tc.tile_pool(name="psB", bufs=2, space="PSUM"); psB = cmB.__enter__()
        for g in range(8):
            ps = psB.tile([128, 4, C], F32, tag="bps", name="bps")
            for jj in range(4):
                j = 4 * g + jj
                for kc in range(2):
                    nc.tensor.matmul(ps[:, jj, :],
                                     xT_cm[kc][:, 128 * j:128 * (j + 1)],
                                     wv_sb[:, kc, :], start=(kc == 0), stop=False)
                nc.tensor.matmul(ps[:, jj, :], onesr[:, 0:128], bv_row[:],
                                 start=False, stop=True)
            for jj in range(4):
                nc.scalar.copy(vT[:, 4 * g + jj, :], ps[:, jj, :])
        cmB.__exit__(None, None, None)
        cmCl.__exit__(None, None, None)
        if debug:
            nc.gpsimd.dma_start(
                d["dbg_vt"].rearrange("p (j c) -> p j c", j=NCHUNK), vT[:])

        # ---- phase C (cont.): per-p row sums via selector matmuls ----
        cmC = tc.tile_pool(name="psCrow", bufs=2, space="PSUM"); psC = cmC.__enter__()
        ROWCH = [(r0, min(7, 64 - r0)) for r0 in range(0, 64, 7)]
        for r0, nr in ROWCH:
            N = nr * 66
            win = slice((r0 + 1) * 66, (r0 + 1) * 66 + N)
            ps = psC.tile([9, 512], F32, tag="sps", name="sps")
            nc.tensor.matmul(ps[:, 0:N], selsum[:], AE[:, win],
                             start=True, stop=True)
            rchf = small.tile([9, 512], F32, tag="rchf", name="rchf", bufs=2)
            nc.vector.reciprocal_approx_fast(rchf[:, 0:N], ps[:, 0:N])
            rch = small.tile([9, 512], F16, tag="rch", name="rch", bufs=2)
            nc.scalar.copy(rch[:, 0:N], rchf[:, 0:N])
            ps2 = psC.tile([81, 512], F32, tag="rps", name="rps")
            nc.tensor.matmul(ps2[:, 0:N], selrep[:], rch[:, 0:N],
                             start=True, stop=True)
            iv = AE3[:, r0 + 1:r0 + 1 + nr, 1:65]
            nc.vector.tensor_tensor(
                iv, iv, ps2[:, 0:N].rearrange("p (r s) -> p r s", s=66)[:, :, 1:65],
                op=ALU.mult)
        cmC.__exit__(None, None, None)

        # ---- phase D: W stencil build (9 shifted selector matmuls) ----
        cmD = tc.tile_pool(name="psD", bufs=1, space="PSUM"); psD = cmD.__enter__()
        W_tap = mid.tile([25, L], F16, tag="wtap", name="W_tap")
        wmask_t = wmask.rearrange("p (u v) -> p v u", u=64)
        wtap_t = W_tap.rearrange("p (u v) -> p v u", u=64)
        for gstart in (0, 5):
            grp = ROWCH[gstart:gstart + 5]
            pss = [psD.tile([25, 512], F32, tag=f"wps{q}", name=f"wps{q}")
                   for q in range(len(grp))]
            for dd, (di, dj) in enumerate(product(range(3), range(3))):
                for q, (r0, nr) in enumerate(grp):
                    N = nr * 66
                    st = (r0 + 2 - dj) * 66 + (2 - di)
                    nc.tensor.matmul(pss[q][:, 0:N],
                                     selshift[:, 25 * dd:25 * (dd + 1)],
                                     AE[:, st:st + N],
                                     start=(dd == 0), stop=(dd == 8))
            for q, (r0, nr) in enumerate(grp):
                N = nr * 66
                nc.vector.tensor_tensor(
                    wtap_t[:, r0:r0 + nr, :],
                    pss[q][:, 0:N].rearrange("p (r s) -> p r s", s=66)[:, :, 0:64],
                    wmask_t[:, r0:r0 + nr, :], op=ALU.mult)
        if debug:
            nc.gpsimd.dma_start(d["dbg_w"][:], W_tap[:])
        cmD.__exit__(None, None, None)
        cmD2 = tc.tile_pool(name="psD2", bufs=2, space="PSUM"); psD2 = cmD2.__enter__()
        W_tm = main.tile([128, NCHUNK, 25], F32, tag="W_tm", name="W_tm")
        for j in range(NCHUNK):
            pt = psD2.tile([128, 25], F16, tag="wtp", name="wtp")
            nc.tensor.transpose(pt[:], W_tap[:, 128 * j:128 * (j + 1)],
                                ident[0:25, 0:25])
            nc.scalar.copy(W_tm[:, j, :], pt[:])
        cmD2.__exit__(None, None, None)
        cmMid.__exit__(None, None, None)

        # ---- maxpools first in the DVE stream (only need xT_cm; they fill
        # the head while W/vd are still being built) ----
        ptmp = es.enter_context(tc.tile_pool(name="ptmp", bufs=1))
        m1 = [main.tile([128, L], F16, tag=f"m1{cc}", name=f"m1_{cc}") for cc in range(2)]
        m2 = [main.tile([128, L], F16, tag=f"m2{cc}", name=f"m2_{cc}") for cc in range(2)]

        def g3(ap):
            return ap.rearrange("p (h w) -> p h w", h=64)

        def hmax3(eng, dst, src):
            dv, sv = g3(dst), g3(src)
            t1 = ptmp.tile([128, L], F16, tag="ptmp", name="ptmp")
            tv = g3(t1)
            eng.tensor_tensor(tv[:, :, 1:], sv[:, :, 1:], sv[:, :, :63], op=ALU.max)
            nc.scalar.copy(tv[:, :, 0:1], sv[:, :, 0:1])
            eng.tensor_tensor(dv[:, :, :63], tv[:, :, :63], sv[:, :, 1:], op=ALU.max)
            nc.scalar.copy(dv[:, :, 63:64], tv[:, :, 63:64])

        def vmax3(eng, dst, src):
            dv, sv = g3(dst), g3(src)
            t1 = ptmp.tile([128, L], F16, tag="ptmp", name="ptmp")
            tv = g3(t1)
            eng.tensor_tensor(tv[:, 1:, :], sv[:, 1:, :], sv[:, :63, :], op=ALU.max)
            nc.scalar.copy(tv[:, 0:1, :], sv[:, 0:1, :])
            eng.tensor_tensor(dv[:, :63, :], tv[:, :63, :], sv[:, 1:, :], op=ALU.max)
            nc.scalar.copy(dv[:, 63:64, :], tv[:, 63:64, :])

        def hspread(eng, dst, src):
            dv, sv = g3(dst), g3(src)
            eng.tensor_tensor(dv[:, :, 1:63], sv[:, :, 0:62], sv[:, :, 2:64], op=ALU.max)
            nc.scalar.copy(dv[:, :, 0:1], sv[:, :, 1:2])
            nc.scalar.copy(dv[:, :, 63:64], sv[:, :, 62:63])

        def vspread(eng, dst, src):
            dv, sv = g3(dst), g3(src)
            eng.tensor_tensor(dv[:, 1:63, :], sv[:, 0:62, :], sv[:, 2:64, :], op=ALU.max)
            nc.scalar.copy(dv[:, 0:1, :], sv[:, 1:2, :])
            nc.scalar.copy(dv[:, 63:64, :], sv[:, 62:63, :])

        for cc in range(2):
            eng = nc.vector
            cm3 = ptmp.tile([128, L], F16, tag="ptmp2", name="ptmp2")
            hmax3(eng, cm3, xT_cm[cc])
            vmax3(eng, m1[cc], cm3)
            cm5 = ptmp.tile([128, L], F16, tag="ptmp3", name="ptmp3")
            hspread(eng, cm5, cm3)
            r35 = ptmp.tile([128, L], F16, tag="ptmp2", name="ptmp2")
            vmax3(eng, r35, cm5)
            vspread(eng, m2[cc], r35)

        # ---- shifted copies of vT via SBUF->SBUF DMA ----
        # S(delta)[p, j] = v[128j + p + delta]; lanes whose source would leave
        # [0, 4096) are clamp-filled with real (finite) data -- their taps have
        # W == 0 via wmask, so any finite value is safe (never NaN).
        def shift_copy(eng, dst, delta):
            dd = abs(delta)
            if delta > 0:
                eng.dma_start(dst[0:128 - dd, :, :], vT[dd:128, :, :])
                eng.dma_start(dst[128 - dd:128, 0:NCHUNK - 1, :],
                              vT[0:dd, 1:NCHUNK, :])
                eng.dma_start(dst[128 - dd:128, NCHUNK - 1, :],
                              vT[0:dd, NCHUNK - 1, :])          # clamp (W=0)
            else:
                eng.dma_start(dst[dd:128, :, :], vT[0:128 - dd, :, :])
                eng.dma_start(dst[0:dd, 1:NCHUNK, :],
                              vT[128 - dd:128, 0:NCHUNK - 1, :])
                eng.dma_start(dst[0:dd, 0, :], vT[128 - dd:128, 0, :])  # clamp

        def vd_tile(tag, nm):
            return main.tile([128, NCHUNK, C], F16, tag=tag, name=nm)

        # A-family first (4 copies), then edges, then B-family. Tag reuse
        # pairs a copy with a later one; the later DMA (sync queue) then
        # waits for the earlier copy's readers, which sit on OTHER queues
        # (vector / tensor) -- never behind the DMA on its own queue.
        A_p2 = vd_tile("vdg0", "A_p2"); shift_copy(nc.scalar, A_p2, 2)
        A_m2 = vd_tile("vdg1", "A_m2"); shift_copy(nc.scalar, A_m2, -2)
        A_p1 = vd_tile("vdd0", "A_p1"); shift_copy(nc.scalar, A_p1, 1)
        A_m1 = vd_tile("vdd1", "A_m1"); shift_copy(nc.scalar, A_m1, -1)

        # edge tiles for tap (e=-1, f) at chunk 0: edgeB[p, fi] = v[p - 64 + f]
        # (valid lanes p >= 64; lanes < 64+|f| have W=0, clamp-filled)
        EDGEF = (0, 1, -1, 2, -2)
        FI = {f: i for i, f in enumerate(EDGEF)}
        edgeB = main.tile([128, 5, C], F16, tag="edgeB", name="edgeB")
        for fi, f in enumerate(EDGEF):
            if f >= 0:
                nc.scalar.dma_start(edgeB[64:128, fi, :], vT[f:64 + f, 0, :])
                nc.scalar.dma_start(edgeB[0:64, fi, :], vT[0:64, 0, :])
            else:
                nc.scalar.dma_start(edgeB[64 - f:128, fi, :], vT[0:64 + f, 0, :])
                nc.scalar.dma_start(edgeB[0:64 - f, fi, :], vT[0:64 - f, 0, :])

        B_0 = vd_tile("vdd0", "B_0"); shift_copy(nc.sync, B_0, 64)
        B_m1 = vd_tile("vdd0", "B_m1"); shift_copy(nc.sync, B_m1, 63)
        B_p1 = vd_tile("vdd1", "B_p1"); shift_copy(nc.sync, B_p1, 65)
        B_p2 = vd_tile("vdg0", "B_p2"); shift_copy(nc.sync, B_p2, 66)
        B_m2 = vd_tile("vdg1", "B_m2"); shift_copy(nc.sync, B_m2, 62)
        BF = {0: B_0, 1: B_p1, -1: B_m1, 2: B_p2, -2: B_m2}

        # ---- phase E: 25-tap apply ----
        # 13 taps as DVE scalar_tensor_tensor FMAs into acc_d; 12 taps on
        # ScalarE+PE: ScalarE builds diag(w) tiles (reads only ident/W_tm),
        # PE accumulates psum[j] += diag(w) @ v_shifted into per-chunk PSUM
        # banks, folding acc_d in at the end.  GPSIMD is useless here -- it
        # contends with DVE's SBUF port and its AP-scalar ops run at ~4us.
        acc_d = main.tile([128, NCHUNK, C], F16, tag="acc", name="acc_d")
        acc2 = main.tile([128, NCHUNK, C], F16, tag="accg", name="acc2")
        VDT = {0: vT, 1: A_p1, -1: A_m1, 2: A_p2, -2: A_m2}

        def tap_sources(e, f):
            """Yield (j, src_ap) for tap (e, f)."""
            if e % 2 == 0:
                vdt, off = VDT[f], e // 2
                for j in range(NCHUNK):
                    jp = j + off
                    if 0 <= jp < NCHUNK:
                        yield j, vdt[:, jp, :]
            elif e == 1:
                bt = BF[f]
                for j in range(NCHUNK):
                    yield j, bt[:, j, :]
            else:
                bt = BF[f]
                yield 0, edgeB[:, FI[f], :]
                for j in range(1, NCHUNK):
                    yield j, bt[:, j - 1, :]

        def dve_fma(j, src, t, first=False):
            wap = W_tm[:, j:j + 1, t:t + 1]
            dst = acc_d[:, j, :]
            if first:
                nc.vector.tensor_scalar(dst, src, wap, None, op0=ALU.mult)
            else:
                nc.vector.scalar_tensor_tensor(dst, src, wap, dst,
                                               op0=ALU.mult, op1=ALU.add)

        # DVE phase 1 (chunk-inner; sources available early)
        for j in range(NCHUNK):
            dve_fma(j, vT[:, j, :], TAPI[(0, 0)], first=True)
        for e, f in ((2, 0), (-2, 0), (0, 1), (-2, 1), (0, -1), (-2, -1)):
            t = TAPI[(e, f)]
            for j, src in tap_sources(e, f):
                dve_fma(j, src, t)
        # DVE phase 2 (tap-major; B-family sources arrive while phase 1 runs)
        for e, f in ((1, 0), (-1, 0), (1, 1), (-1, 1), (1, -1), (-1, -1)):
            t = TAPI[(e, f)]
            for j, src in tap_sources(e, f):
                dve_fma(j, src, t)

        # ScalarE+PE path
        dtp = es.enter_context(tc.tile_pool(name="dtp", bufs=8))
        cmE = tc.tile_pool(name="psE", bufs=1, space="PSUM"); psE = cmE.__enter__()
        GRP = 8
        SWEEP_A = [(2, 1), (2, -1), (0, 2), (2, 2), (-2, 2),
                   (0, -2), (2, -2), (-2, -2)]
        SWEEP_B = [(1, 2), (-1, 2), (1, -2), (-1, -2)]

        def pe_sweep(taps, fold_acc2, fold_accd):
            for g0 in range(0, NCHUNK, GRP):
                ops = {j: [] for j in range(g0, g0 + GRP)}
                for e, f in taps:
                    t = TAPI[(e, f)]
                    for j, src in tap_sources(e, f):
                        if g0 <= j < g0 + GRP:
                            ops[j].append((t, src))
                for j in range(g0, g0 + GRP):
                    if fold_acc2:
                        ops[j].append((None, acc2[:, j, :]))
                    if fold_accd:
                        ops[j].append((None, acc_d[:, j, :]))
                pss = {j: psE.tile([128, 512], F32, tag=f"eps{j - g0}",
                                   name=f"eps{j - g0}")
                       for j in range(g0, g0 + GRP)}
                for j in range(g0, g0 + GRP):
                    n = len(ops[j])
                    for k, (t, src) in enumerate(ops[j]):
                        if t is None:
                            lhs = ident[:]
                        else:
                            dt = dtp.tile([128, 128], F16, tag="dt", name="dt")
                            nc.scalar.activation(dt[:], ident[:], ACT.Copy,
                                                 scale=W_tm[:, j:j + 1, t:t + 1])
                            lhs = dt[:]
                        nc.tensor.matmul(pss[j][:, 0:C], lhs, src,
                                         start=(k == 0), stop=(k == n - 1))
                    nc.scalar.copy(acc2[:, j, :], pss[j][:, 0:C])

        # sweep A: 8 even-e taps -> acc2 (runs alongside DVE phase 1)
        pe_sweep(SWEEP_A, fold_acc2=False, fold_accd=False)
        # sweep B: 4 odd-e taps + acc2 -> acc2 (independent of acc_d)
        pe_sweep(SWEEP_B, fold_acc2=True, fold_accd=False)
        cmE.__exit__(None, None, None)
        # final merge on DVE right after its last tap (fp16 2x mode)
        for j in range(NCHUNK):
            nc.vector.tensor_tensor(acc_d[:, j, :], acc_d[:, j, :],
                                    acc2[:, j, :], op=ALU.add)
        if debug:
            nc.gpsimd.dma_start(
                d["dbg_acc"].rearrange("p (j c) -> p j c", j=NCHUNK), acc_d[:])

        # ---- phase G: xf transpose-evac + relu/maxpool chain ----
        # x1 = relu(relu(xfT) + m1^T); x2 = relu(x1 + m2^T)  (x2 in-place in m2;
        # x1 reuses the DVE vd slots, which are dead after phase E)
        cmG = tc.tile_pool(name="psG", bufs=4, space="PSUM"); psG = cmG.__enter__()
        x1 = [main.tile([128, L], F16, tag=f"vdd{cc}", name=f"x1_{cc}")
              for cc in range(2)]
        for j2 in range(NCHUNK // 2):
            for cc in range(2):
                pt = psG.tile([128, 2, 128], F16, tag="tp", name="tp")
                for u in range(2):
                    nc.tensor.transpose(
                        pt[:, u, :],
                        acc_d[:, 2 * j2 + u, 128 * cc:128 * (cc + 1)], ident[:])
                nc.scalar.activation(x1[cc][:, 256 * j2:256 * (j2 + 1)],
                                     pt.rearrange("p a b -> p (a b)"), ACT.Relu)
        cmG.__exit__(None, None, None)
        x2 = m2
        for cc in range(2):
            nc.vector.tensor_tensor(x1[cc][:], x1[cc][:], m1[cc][:], op=ALU.add)
            nc.scalar.activation(x1[cc][:], x1[cc][:], ACT.Relu)
            nc.vector.tensor_tensor(x2[cc][:], x1[cc][:], m2[cc][:], op=ALU.add)
            nc.scalar.activation(x2[cc][:], x2[cc][:], ACT.Relu)
        if debug:
            for cc in range(2):
                nc.gpsimd.dma_start(d["dbg_x1"][128 * cc:128 * (cc + 1), :], x1[cc][:])

        # ---- phase H: fu matmul + bias + relu + residual, incremental BN ----
        cmH = tc.tile_pool(name="psH", bufs=2, space="PSUM"); psH = cmH.__enter__()
        out_all = main.tile([128, 2, L], F16, tag="acc", name="out_all")
        out_cm = [out_all[:, cc, :] for cc in range(2)]
        st = small.tile([128, 2, 8, 6], F32, tag="bnst", name="bnst")
        rhss = [x1[0], x1[1], x2[0], x2[1]]
        for mc in range(2):
            for half in range(2):
                ps = psH.tile([128, 4, 512], F32, tag="fups", name="fups")
                for q in range(4):
                    n8 = 4 * half + q
                    for kc in range(4):
                        nc.tensor.matmul(ps[:, q, :], wfu_sb[:, kc, mc, :],
                                         rhss[kc][:, 512 * n8:512 * (n8 + 1)],
                                         start=(kc == 0), stop=False)
                    nc.tensor.matmul(ps[:, q, :],
                                     bfu_row[:, 128 * mc:128 * (mc + 1)],
                                     onesr[:], start=False, stop=True)
                for q in range(4):
                    n8 = 4 * half + q
                    sl = slice(512 * n8, 512 * (n8 + 1))
                    nc.scalar.activation(out_cm[mc][:, sl], ps[:, q, :], ACT.Relu)
                    nc.vector.tensor_tensor(out_cm[mc][:, sl], out_cm[mc][:, sl],
                                            xT_cm[mc][:, sl], op=ALU.add)
                    nc.vector.bn_stats(st[:, mc, n8, :], out_cm[mc][:, sl])
        cmH.__exit__(None, None, None)

        # ---- BN: pack local sums, single AllReduce, normalize ----
        bnpack = small.tile([128, 4], F32, tag="bnpack", name="bnpack")
        for mc in range(2):
            ag = small.tile([128, 2], F32, tag="bnag", name="bnag", bufs=2)
            nc.vector.bn_aggr(ag[:], st[:, mc])
            nc.vector.tensor_scalar(bnpack[:, 2 * mc:2 * mc + 1], ag[:, 0:1],
                                    float(L), None, op0=ALU.mult)
            sq = small.tile([128, 1], F32, tag="bnsq", name="bnsq", bufs=2)
            nc.vector.tensor_tensor(sq[:], ag[:, 0:1], ag[:, 0:1], op=ALU.mult)
            nc.vector.tensor_tensor(sq[:], sq[:], ag[:, 1:2], op=ALU.add)
            nc.vector.tensor_scalar(bnpack[:, 2 * mc + 1:2 * mc + 2], sq[:],
                                    float(L), None, op0=ALU.mult)
        cin = dram.tile([128, 4], F32, name="cin")
        cout = dram.tile([128, 4], F32, name="cout")
        nc.sync.dma_start(cin[:], bnpack[:])
        nc.gpsimd.collective_compute(
            "AllReduce", ALU.add, replica_groups=[list(range(n_cores))],
            ins=[cin.opt()], outs=[cout.opt()])
        gs = small.tile([128, 4], F32, tag="gs", name="gs")
        nc.sync.dma_start(gs[:], cout[:])

        NTOT = float(n_cores * L)
        scale = small.tile([128, 2], F32, tag="scale", name="scale")
        shift = small.tile([128, 2], F32, tag="shift", name="shift")
        mean = small.tile([128, 2], F32, tag="mean", name="mean")
        var = small.tile([128, 2], F32, tag="var", name="var")
        for cc in range(2):
            nc.vector.tensor_scalar(mean[:, cc:cc + 1], gs[:, 2 * cc:2 * cc + 1],
                                    1.0 / NTOT, None, op0=ALU.mult)
            nc.vector.tensor_scalar(var[:, cc:cc + 1], gs[:, 2 * cc + 1:2 * cc + 2],
                                    1.0 / NTOT, None, op0=ALU.mult)
        msq = small.tile([128, 2], F32, tag="msq", name="msq")
        nc.vector.tensor_tensor(msq[:], mean[:], mean[:], op=ALU.mult)
        nc.vector.tensor_tensor(var[:], var[:], msq[:], op=ALU.subtract)
        rs = small.tile([128, 2], F32, tag="rs", name="rs")
        nc.vector.tensor_scalar(var[:], var[:], float(EPS), None, op0=ALU.add)
        nc.scalar.activation(rs[:], var[:], ACT.Sqrt)
        nc.vector.reciprocal(rs[:], rs[:])
        nc.vector.tensor_tensor(scale[:], gamma2[:], rs[:], op=ALU.mult)
        nc.vector.tensor_tensor(shift[:], mean[:], scale[:], op=ALU.mult)
        nc.vector.tensor_tensor(shift[:], beta2[:], shift[:], op=ALU.subtract)

        # normalize in place (fp16 4x), DMA out channel-major fp16
        for cc in range(2):
            for hh in range(2):
                sl = slice(2048 * hh, 2048 * (hh + 1))
                nc.vector.tensor_scalar(out_cm[cc][:, sl], out_cm[cc][:, sl],
                                        scale[:, cc:cc + 1], shift[:, cc:cc + 1],
                                        op0=ALU.mult, op1=ALU.add)
                nc.sync.dma_start(d["y"][128 * cc:128 * (cc + 1), sl],
                                  out_cm[cc][:, sl])


_CACHE = {}


def _get_program(n_cores=N_CORES, debug=False):
    key = (n_cores, debug)
    if key not in _CACHE:
        nc = bacc.Bacc("TRN2", target_bir_lowering=False, debug=False,
                       num_devices=n_cores)
        build(nc, n_cores, debug)
        nc.compile()
        _CACHE[key] = nc
    return _CACHE[key]


def make_in_map(inputs, b):
    consts = host_consts()
    f16 = np.float16
    # host-side grid permutation: xf16[u*64+v, c] = x[v, u, c]
    xf16 = np.ascontiguousarray(
        np.asarray(inputs["x"][b]).transpose(1, 0, 2).reshape(L, C)).astype(f16)
    wv = np.ascontiguousarray(
        np.asarray(inputs["Wv"], np.float32).reshape(2, 128, C)
        .transpose(1, 0, 2).reshape(128, 2 * C)).astype(f16)
    wa = np.ascontiguousarray(
        np.asarray(inputs["Wa"], np.float32).reshape(2, 128, 81)
        .transpose(1, 0, 2).reshape(128, 2 * 81)).astype(f16)
    wfu = np.ascontiguousarray(
        np.asarray(inputs["Wfu"], np.float32).reshape(4, 128, 2, 128)
        .transpose(1, 0, 2, 3).reshape(128, 4 * 2 * 128)).astype(f16)
    return {
        "xf16": xf16,
        "wv": wv, "wa": wa, "wfu": wfu,
        "bvrow": np.asarray(inputs["bv"], np.float32).reshape(1, C).astype(f16),
        "barow": np.ascontiguousarray(
            np.asarray(inputs["ba"], np.float32).reshape(81, 1)),
        "bfurow": np.asarray(inputs["bfu"], np.float32).reshape(1, C).astype(f16),
        "gamma2": np.ascontiguousarray(
            np.asarray(inputs["gamma"], np.float32).reshape(2, 128).T),
        "beta2": np.ascontiguousarray(
            np.asarray(inputs["beta"], np.float32).reshape(2, 128).T),
        **consts,
    }


def gather_out(res_y):
    # y[c, u*64+v] -> out[u, v, c], cast fp16 -> f32 on host
    return np.asarray(res_y, dtype=np.float32).reshape(C, H, W).transpose(1, 2, 0)


def kernel(**inputs):
    nc = _get_program()
    in_maps = [make_in_map(inputs, b) for b in range(B)]
    res = run_bass_kernel_spmd(nc, in_maps, list(range(N_CORES)))
    out = np.stack([gather_out(res.results[b]["y"]) for b in range(B)])
    return out.astype(np.float32)


# revision 24
# speedup vs baseline: 1.2666x; 1.2666x over previous
"""Trainium2 Bass kernel for nn_MOA_13254269075617 (sparse windowed attention block).

Sharding: data-parallel over batch B=8 across 8 NeuronCores (1 image each).
BatchNorm uses global batch stats via one on-device AllReduce of per-channel
sum / sum-of-squares (plus an early warm-up collective to absorb CC-stream
startup cost).

Per-core pipeline (all in the spatially-TRANSPOSED frame T(z)[u,v]=z[v,u],
fp16 end-to-end so DVE elementwise ops run in 2x/4x perf modes):
  xT_cm  : x in channel-major [256, 4096], host-pre-permuted grid, loaded
           via 4 big transpose-DMAs
  vT     : (xT @ Wv + bv) token-major [128, 32, 256] fp16
  AE     : exp-logits pq-major [81, 4096] on a zero-padded 66-pitch grid
  W      : 25-tap position-varying stencil weights (fold+attention combined)
  vd     : 9 token-shifted copies of vT via SBUF->SBUF DMA (no PE work)
  acc    : 25-tap stencil apply, token-major fp16 FMAs on DVE (2x mode)
  x1/x2  : relu chains with 3x3/5x5 maxpools (separable shifted-max trees)
  out    : concat-matmul (Wfu) + residual, BN with AllReduce'd stats
  y      : channel-major fp16 output, host casts/transposes to f32 [H,W,C]
"""
import sys

for _p in (
    "/root/.axon_site",
    "/root/.axon_site/_ro/trn_rl_repo",
    "/root/.axon_site/_ro/pypackages",
    "/opt/trn_rl_repo",
):
    if _p not in sys.path:
        sys.path.append(_p)

from itertools import product

import numpy as np

import concourse.bass as bass
import concourse.tile as tile
from concourse import bacc, mybir
from concourse.bass_utils import run_bass_kernel_spmd

F32 = mybir.dt.float32
F16 = mybir.dt.float16
ALU = mybir.AluOpType
ACT = mybir.ActivationFunctionType

B, H, W, C = 8, 64, 64, 256
L = H * W                      # 4096 tokens
NCHUNK = L // 128              # 32 token chunks
N_CORES = 8
EPS = 1e-5

# (e, f) tap -> t index and source shift delta = 64*e + f
TAPI = {(e, f): (e + 2) * 5 + (f + 2) for e in range(-2, 3) for f in range(-2, 3)}


def host_consts():
    """Selector matrices and small constants (host-precomputed, same all cores)."""
    selsum = np.zeros((81, 9), np.float32)
    for p in range(9):
        selsum[9 * p:9 * p + 9, p] = 1.0
    selrep = np.zeros((9, 81), np.float32)
    for p in range(9):
        selrep[p, 9 * p:9 * p + 9] = 1.0
    # selshift[:, 25*d + tap]: for (di,dj) block d, tap (e,f):
    #   k = 9*(3di+dj) + 3(di+e)+(dj+f) if di+e,dj+f in [0,3)
    selshift = np.zeros((81, 9 * 25), np.float32)
    for d, (di, dj) in enumerate(product(range(3), range(3))):
        for t, (e, f) in enumerate(product(range(-2, 3), range(-2, 3))):
            dip, djp = di + e, dj + f
            if 0 <= dip < 3 and 0 <= djp < 3:
                k = 9 * (3 * di + dj) + (3 * dip + djp)
                selshift[k, 25 * d + t] = 1.0
    wmask = np.ones((25, 64, 64), np.float32)
    for t, (e, f) in enumerate(product(range(-2, 3), range(-2, 3))):
        if e > 0: wmask[t, 64 - e:, :] = 0
        if e < 0: wmask[t, :-e, :] = 0
        if f > 0: wmask[t, :, 64 - f:] = 0
        if f < 0: wmask[t, :, :-f] = 0
    f16 = np.float16
    return {
        "selsum": selsum.astype(f16),
        "selrep": selrep.astype(f16),
        "selshift": selshift.astype(f16),
        "wmask": wmask.reshape(25, 4096).astype(f16),
        "ident": np.eye(128, dtype=f16),
        "onesr": np.ones((1, 512), f16),
    }


def build(nc, n_cores, debug=False):
    d = {}
    def din(name, shape, dt=F16):
        d[name] = nc.dram_tensor(name, list(shape), dt, kind="ExternalInput").ap()
    def dout(name, shape, dt=F16):
        d[name] = nc.dram_tensor(name, list(shape), dt, kind="ExternalOutput").ap()

    din("xf16", (L, C))
    din("wv", (128, 2 * C)); din("bvrow", (1, C))
    din("wa", (128, 2 * 81)); din("barow", (81, 1), F32)
    din("wfu", (128, 4 * 2 * 128)); din("bfurow", (1, C))
    din("gamma2", (128, 2), F32); din("beta2", (128, 2), F32)
    din("selsum", (81, 9)); din("selrep", (9, 81)); din("selshift", (81, 225))
    din("ident", (128, 128)); din("onesr", (1, 512)); din("wmask", (25, L))
    dout("y", (2 * 128, L))
    if debug:
        dout("dbg_vt", (128, NCHUNK * C))
        dout("dbg_w", (25, L))
        dout("dbg_acc", (128, NCHUNK * C))
        dout("dbg_x1", (2 * 128, L))

    with tile.TileContext(nc) as tc:
        _build_tc(tc, d, n_cores, debug)
    return d


def _build_tc(tc, d, n_cores, debug):
    nc = tc.nc
    from contextlib import ExitStack
    es = ExitStack()
    with es:
        consts = es.enter_context(tc.tile_pool(name="consts", bufs=1))
        main = es.enter_context(tc.tile_pool(name="main", bufs=1))
        small = es.enter_context(tc.tile_pool(name="small", bufs=1))
        dram = es.enter_context(tc.tile_pool(name="dram", bufs=2, space="DRAM"))

        # ---- phase A: input loads first (run under the CC entry barrier) ----
        xT_cm = [main.tile([128, L], F16, tag=f"xcm{cc}", name=f"xT_cm{cc}")
                 for cc in range(2)]
        for cc in range(2):
            for q in range(2):
                nc.sync.dma_start_transpose(
                    xT_cm[cc][:, 2048 * q:2048 * (q + 1)],
                    d["xf16"][2048 * q:2048 * (q + 1), 128 * cc:128 * (cc + 1)])

        # ---- const loads on the scalar HWDGE queue ----
        def cload(name, shape, dt=F16, src=None):
            t = consts.tile(list(shape), dt, tag=name, name=name)
            nc.scalar.dma_start(t[:], (src if src is not None else d[name])[:])
            return t
        ident = cload("ident", (128, 128))
        onesr = cload("onesr", (1, 512))
        bv_row = cload("bvrow", (1, C))
        bfu_row = cload("bfurow", (1, C))
        ba_sb = cload("barow", (81, 1), F32)
        selsum = cload("selsum", (81, 9))
        selrep = cload("selrep", (9, 81))
        selshift = cload("selshift", (81, 225))
        gamma2 = cload("gamma2", (128, 2), F32)
        beta2 = cload("beta2", (128, 2), F32)
        wv_sb = cload("wv", (128, 2, C), src=d["wv"].rearrange("p (k c) -> p k c", k=2))
        wa_sb = cload("wa", (128, 2, 81), src=d["wa"].rearrange("p (k c) -> p k c", k=2))
        wfu_sb = cload("wfu", (128, 4, 2, 128),
                       src=d["wfu"].rearrange("p (k m c) -> p k m c", k=4, m=2))

        # ---- warm-up collective: absorbs CC stream startup (~20us) early ----
        warm_in = dram.tile([1, 1], F32, name="warm_in")
        warm_out = dram.tile([1, 1], F32, name="warm_out")
        nc.sync.dma_start(warm_in[:], ba_sb[0:1, 0:1])
        nc.gpsimd.collective_compute(
            "AllReduce", ALU.add, replica_groups=[list(range(n_cores))],
            ins=[warm_in.opt()], outs=[warm_out.opt()])

        # ---- phase C: attention logits -> exp -> normalize ----
        # AE grid: (g1=u, g2=v); AE[g1+1, g2+1] = softmax-numerator of the
        # ORIGINAL position (h=g2, w=g1) (x transposed-grid ordering).
        cmMid = tc.tile_pool(name="mid", bufs=1); mid = cmMid.__enter__()
        wmask = mid.tile([25, L], F16, tag="wmask", name="wmask")
        nc.scalar.dma_start(wmask[:], d["wmask"][:])
        cmCl = tc.tile_pool(name="psClog", bufs=1, space="PSUM"); psCl = cmCl.__enter__()
        AE = mid.tile([81, 66 * 67], F16, tag="AE", name="AE")
        AE3 = AE.rearrange("p (r s) -> p r s", r=67)
        nc.vector.memset(AE3[:, 0:1, :], 0.0)          # pad ring only; the
        nc.vector.memset(AE3[:, 65:67, :], 0.0)        # 64x64 interior is
        nc.vector.memset(AE3[:, 1:65, 0:1], 0.0)       # fully written by exp
        nc.vector.memset(AE3[:, 1:65, 65:66], 0.0)

        for grp in range(2):
            pss = [psCl.tile([81, 512], F32, tag=f"aps{q}", name=f"aps{q}")
                   for q in range(4)]
            for kc in range(2):
                for q in range(4):
                    n8 = 4 * grp + q
                    nc.tensor.matmul(pss[q][:], wa_sb[:, kc, :],
                                     xT_cm[kc][:, 512 * n8:512 * (n8 + 1)],
                                     start=(kc == 0), stop=(kc == 1))
            for q in range(4):
                n8 = 4 * grp + q
                nc.scalar.activation(AE3[:, 1 + 8 * n8:1 + 8 * n8 + 8, 1:65],
                                     pss[q].rearrange("p (r s) -> p r s", s=64),
                                     ACT.Exp, bias=ba_sb[:, 0:1])

        # ---- phase B: vT = xT @ Wv + bv, token-major fp16 ----
        # (emitted between C-logits and C-rowsums so vT lands early for the
        # shifted-copy DMAs; PE queue order = emission order)
        vT = main.tile([128, NCHUNK, C], F16, tag="vT", name="vT")
        cmB = tc.tile_pool(name="psB", bufs=2, space="PSUM"); psB = cmB.__enter__()
        for g in range(8):
            ps = psB.tile([128, 4, C], F32, tag="bps", name="bps")
            for jj in range(4):
                j = 4 * g + jj
                for kc in range(2):
                    nc.tensor.matmul(ps[:, jj, :],
                                     xT_cm[kc][:, 128 * j:128 * (j + 1)],
                                     wv_sb[:, kc, :], start=(kc == 0), stop=False)
                nc.tensor.matmul(ps[:, jj, :], onesr[:, 0:128], bv_row[:],
                                 start=False, stop=True)
            for jj in range(4):
                nc.scalar.copy(vT[:, 4 * g + jj, :], ps[:, jj, :])
        cmB.__exit__(None, None, None)
        cmCl.__exit__(None, None, None)
        if debug:
            nc.gpsimd.dma_start(
                d["dbg_vt"].rearrange("p (j c) -> p j c", j=NCHUNK), vT[:])

        # ---- phase C (cont.): per-p row sums via selector matmuls ----
        cmC = tc.tile_pool(name="psCrow", bufs=2, space="PSUM"); psC = cmC.__enter__()
        ROWCH = [(r0, min(7, 64 - r0)) for r0 in range(0, 64, 7)]
        for r0, nr in ROWCH:
            N = nr * 66
            win = slice((r0 + 1) * 66, (r0 + 1) * 66 + N)
            ps = psC.tile([9, 512], F32, tag="sps", name="sps")
            nc.tensor.matmul(ps[:, 0:N], selsum[:], AE[:, win],
                             start=True, stop=True)
            rchf = small.tile([9, 512], F32, tag="rchf", name="rchf", bufs=2)
            nc.vector.reciprocal_approx_fast(rchf[:, 0:N], ps[:, 0:N])
            rch = small.tile([9, 512], F16, tag="rch", name="rch", bufs=2)
            nc.scalar.copy(rch[:, 0:N], rchf[:, 0:N])
            ps2 = psC.tile([81, 512], F32, tag="rps", name="rps")
            nc.tensor.matmul(ps2[:, 0:N], selrep[:], rch[:, 0:N],
                             start=True, stop=True)
            rt = small.tile([81, 512], F16, tag="rt", name="rt", bufs=2)
            nc.scalar.copy(rt[:, 0:N], ps2[:, 0:N])
            iv = AE3[:, r0 + 1:r0 + 1 + nr, 1:65]
            nc.vector.tensor_tensor(
                iv, iv, rt[:, 0:N].rearrange("p (r s) -> p r s", s=66)[:, :, 1:65],
                op=ALU.mult)
        cmC.__exit__(None, None, None)

        # ---- phase D: W stencil build (9 shifted selector matmuls) ----
        cmD = tc.tile_pool(name="psD", bufs=1, space="PSUM"); psD = cmD.__enter__()
        W_tap = mid.tile([25, L], F16, tag="wtap", name="W_tap")
        wmask_t = wmask.rearrange("p (u v) -> p v u", u=64)
        wtap_t = W_tap.rearrange("p (u v) -> p v u", u=64)
        for gstart in (0, 5):
            grp = ROWCH[gstart:gstart + 5]
            pss = [psD.tile([25, 512], F32, tag=f"wps{q}", name=f"wps{q}")
                   for q in range(len(grp))]
            for dd, (di, dj) in enumerate(product(range(3), range(3))):
                for q, (r0, nr) in enumerate(grp):
                    N = nr * 66
                    st = (r0 + 2 - dj) * 66 + (2 - di)
                    nc.tensor.matmul(pss[q][:, 0:N],
                                     selshift[:, 25 * dd:25 * (dd + 1)],
                                     AE[:, st:st + N],
                                     start=(dd == 0), stop=(dd == 8))
            for q, (r0, nr) in enumerate(grp):
                N = nr * 66
                wt = small.tile([25, 512], F16, tag="wt", name="wt", bufs=2)
                nc.scalar.copy(wt[:, 0:N], pss[q][:, 0:N])
                nc.vector.tensor_tensor(
                    wtap_t[:, r0:r0 + nr, :],
                    wt[:, 0:N].rearrange("p (r s) -> p r s", s=66)[:, :, 0:64],
                    wmask_t[:, r0:r0 + nr, :], op=ALU.mult)
        if debug:
            nc.gpsimd.dma_start(d["dbg_w"][:], W_tap[:])
        cmD.__exit__(None, None, None)
        cmD2 = tc.tile_pool(name="psD2", bufs=2, space="PSUM"); psD2 = cmD2.__enter__()
        W_tm = main.tile([128, NCHUNK, 25], F32, tag="W_tm", name="W_tm")
        for j in range(NCHUNK):
            pt = psD2.tile([128, 25], F16, tag="wtp", name="wtp")
            nc.tensor.transpose(pt[:], W_tap[:, 128 * j:128 * (j + 1)],
                                ident[0:25, 0:25])
            nc.scalar.copy(W_tm[:, j, :], pt[:])
        cmD2.__exit__(None, None, None)
        cmMid.__exit__(None, None, None)

        # ---- maxpools first in the DVE stream (only need xT_cm; they fill
        # the head while W/vd are still being built) ----
        ptmp = es.enter_context(tc.tile_pool(name="ptmp", bufs=1))
        m1 = [main.tile([128, L], F16, tag=f"m1{cc}", name=f"m1_{cc}") for cc in range(2)]
        m2 = [main.tile([128, L], F16, tag=f"m2{cc}", name=f"m2_{cc}") for cc in range(2)]

        def g3(ap):
            return ap.rearrange("p (h w) -> p h w", h=64)

        def hmax3(eng, dst, src):
            dv, sv = g3(dst), g3(src)
            t1 = ptmp.tile([128, L], F16, tag="ptmp", name="ptmp")
            tv = g3(t1)
            eng.tensor_tensor(tv[:, :, 1:], sv[:, :, 1:], sv[:, :, :63], op=ALU.max)
            nc.scalar.copy(tv[:, :, 0:1], sv[:, :, 0:1])
            eng.tensor_tensor(dv[:, :, :63], tv[:, :, :63], sv[:, :, 1:], op=ALU.max)
            nc.scalar.copy(dv[:, :, 63:64], tv[:, :, 63:64])

        def vmax3(eng, dst, src):
            dv, sv = g3(dst), g3(src)
            t1 = ptmp.tile([128, L], F16, tag="ptmp", name="ptmp")
            tv = g3(t1)
            eng.tensor_tensor(tv[:, 1:, :], sv[:, 1:, :], sv[:, :63, :], op=ALU.max)
            nc.scalar.copy(tv[:, 0:1, :], sv[:, 0:1, :])
            eng.tensor_tensor(dv[:, :63, :], tv[:, :63, :], sv[:, 1:, :], op=ALU.max)
            nc.scalar.copy(dv[:, 63:64, :], tv[:, 63:64, :])

        def hspread(eng, dst, src):
            dv, sv = g3(dst), g3(src)
            eng.tensor_tensor(dv[:, :, 1:63], sv[:, :, 0:62], sv[:, :, 2:64], op=ALU.max)
            nc.scalar.copy(dv[:, :, 0:1], sv[:, :, 1:2])
            nc.scalar.copy(dv[:, :, 63:64], sv[:, :, 62:63])

        def vspread(eng, dst, src):
            dv, sv = g3(dst), g3(src)
            eng.tensor_tensor(dv[:, 1:63, :], sv[:, 0:62, :], sv[:, 2:64, :], op=ALU.max)
            nc.scalar.copy(dv[:, 0:1, :], sv[:, 1:2, :])
            nc.scalar.copy(dv[:, 63:64, :], sv[:, 62:63, :])

        for cc in range(2):
            eng = nc.vector
            cm3 = ptmp.tile([128, L], F16, tag="ptmp2", name="ptmp2")
            hmax3(eng, cm3, xT_cm[cc])
            vmax3(eng, m1[cc], cm3)
            cm5 = ptmp.tile([128, L], F16, tag="ptmp3", name="ptmp3")
            hspread(eng, cm5, cm3)
            r35 = ptmp.tile([128, L], F16, tag="ptmp2", name="ptmp2")
            vmax3(eng, r35, cm5)
            vspread(eng, m2[cc], r35)

        # ---- shifted copies of vT via SBUF->SBUF DMA ----
        # S(delta)[p, j] = v[128j + p + delta]; lanes whose source would leave
        # [0, 4096) are clamp-filled with real (finite) data -- their taps have
        # W == 0 via wmask, so any finite value is safe (never NaN).
        def shift_copy(eng, dst, delta):
            dd = abs(delta)
            if delta > 0:
                eng.dma_start(dst[0:128 - dd, :, :], vT[dd:128, :, :])
                eng.dma_start(dst[128 - dd:128, 0:NCHUNK - 1, :],
                              vT[0:dd, 1:NCHUNK, :])
                eng.dma_start(dst[128 - dd:128, NCHUNK - 1, :],
                              vT[0:dd, NCHUNK - 1, :])          # clamp (W=0)
            else:
                eng.dma_start(dst[dd:128, :, :], vT[0:128 - dd, :, :])
                eng.dma_start(dst[0:dd, 1:NCHUNK, :],
                              vT[128 - dd:128, 0:NCHUNK - 1, :])
                eng.dma_start(dst[0:dd, 0, :], vT[128 - dd:128, 0, :])  # clamp

        def vd_tile(tag, nm):
            return main.tile([128, NCHUNK, C], F16, tag=tag, name=nm)

        # A-family first, then edges, then the DVE-side B-family. Tag-reuse
        # WAR waits are always satisfied by readers on OTHER queues.
        # (B_p2/B_m2 are emitted later, between the two PE sweeps.)
        A_p1 = vd_tile("vdd0", "A_p1"); shift_copy(nc.sync, A_p1, 1)
        A_p2 = vd_tile("vdg0", "A_p2"); shift_copy(nc.scalar, A_p2, 2)
        A_m1 = vd_tile("vdd1", "A_m1"); shift_copy(nc.sync, A_m1, -1)
        A_m2 = vd_tile("vdg1", "A_m2"); shift_copy(nc.scalar, A_m2, -2)

        # edge tiles for tap (e=-1, f) at chunk 0: edgeB[p, fi] = v[p - 64 + f]
        # (valid lanes p >= 64; lanes < 64+|f| have W=0, clamp-filled)
        EDGEF = (0, 1, -1, 2, -2)
        FI = {f: i for i, f in enumerate(EDGEF)}
        edgeB = main.tile([128, 5, C], F16, tag="edgeB", name="edgeB")
        for fi, f in enumerate(EDGEF):
            if f >= 0:
                nc.sync.dma_start(edgeB[64:128, fi, :], vT[f:64 + f, 0, :])
                nc.sync.dma_start(edgeB[0:64, fi, :], vT[0:64, 0, :])
            else:
                nc.sync.dma_start(edgeB[64 - f:128, fi, :], vT[0:64 + f, 0, :])
                nc.sync.dma_start(edgeB[0:64 - f, fi, :], vT[0:64 - f, 0, :])

        B_0 = vd_tile("vdd0", "B_0"); shift_copy(nc.sync, B_0, 64)
        B_p1 = vd_tile("vdd1", "B_p1"); shift_copy(nc.sync, B_p1, 65)
        B_m1 = vd_tile("vdd0", "B_m1"); shift_copy(nc.sync, B_m1, 63)
        BF = {0: B_0, 1: B_p1, -1: B_m1}

        # ---- phase E: 25-tap apply ----
        # 13 taps as DVE scalar_tensor_tensor FMAs into acc_d; 12 taps on
        # ScalarE+PE: ScalarE builds diag(w) tiles (reads only ident/W_tm),
        # PE accumulates psum[j] += diag(w) @ v_shifted into per-chunk PSUM
        # banks, folding acc_d in at the end.  GPSIMD is useless here -- it
        # contends with DVE's SBUF port and its AP-scalar ops run at ~4us.
        acc_d = main.tile([128, NCHUNK, C], F16, tag="acc", name="acc_d")
        acc2 = main.tile([128, NCHUNK, C], F16, tag="accg", name="acc2")
        VDT = {0: vT, 1: A_p1, -1: A_m1, 2: A_p2, -2: A_m2}

        def tap_sources(e, f):
            """Yield (j, src_ap) for tap (e, f)."""
            if e % 2 == 0:
                vdt, off = VDT[f], e // 2
                for j in range(NCHUNK):
                    jp = j + off
                    if 0 <= jp < NCHUNK:
                        yield j, vdt[:, jp, :]
            elif e == 1:
                bt = BF[f]
                for j in range(NCHUNK):
                    yield j, bt[:, j, :]
            else:
                bt = BF[f]
                yield 0, edgeB[:, FI[f], :]
                for j in range(1, NCHUNK):
                    yield j, bt[:, j - 1, :]

        def dve_fma(j, src, t, first=False):
            wap = W_tm[:, j:j + 1, t:t + 1]
            dst = acc_d[:, j, :]
            if first:
                nc.vector.tensor_scalar(dst, src, wap, None, op0=ALU.mult)
            else:
                nc.vector.scalar_tensor_tensor(dst, src, wap, dst,
                                               op0=ALU.mult, op1=ALU.add)

        # DVE phase 1 (chunk-inner; sources available early)
        for j in range(NCHUNK):
            dve_fma(j, vT[:, j, :], TAPI[(0, 0)], first=True)
        for e, f in ((2, 0), (-2, 0), (0, 1), (2, 1), (-2, 1),
                     (0, -1), (2, -1), (-2, -1)):
            t = TAPI[(e, f)]
            for j, src in tap_sources(e, f):
                dve_fma(j, src, t)
        # DVE phase 2 (tap-major; B-family sources arrive while phase 1 runs)
        for e, f in ((1, 0), (-1, 0), (1, 1), (-1, 1), (1, -1), (-1, -1)):
            t = TAPI[(e, f)]
            for j, src in tap_sources(e, f):
                dve_fma(j, src, t)

        # ScalarE+PE path
        dtp = es.enter_context(tc.tile_pool(name="dtp", bufs=4))
        cmE = tc.tile_pool(name="psE", bufs=1, space="PSUM"); psE = cmE.__enter__()
        GRP = 8
        SWEEP_A = [(0, 2), (2, 2), (-2, 2), (0, -2), (2, -2), (-2, -2)]
        SWEEP_B = [(1, 2), (-1, 2), (1, -2), (-1, -2)]

        def pe_sweep(taps, fold_acc2, fold_accd):
            for g0 in range(0, NCHUNK, GRP):
                ops = {j: [] for j in range(g0, g0 + GRP)}
                for e, f in taps:
                    t = TAPI[(e, f)]
                    for j, src in tap_sources(e, f):
                        if g0 <= j < g0 + GRP:
                            ops[j].append((t, src))
                for j in range(g0, g0 + GRP):
                    if fold_acc2:
                        ops[j].append((None, acc2[:, j, :]))
                    if fold_accd:
                        ops[j].append((None, acc_d[:, j, :]))
                pss = {j: psE.tile([128, 512], F32, tag=f"eps{j - g0}",
                                   name=f"eps{j - g0}")
                       for j in range(g0, g0 + GRP)}
                for j in range(g0, g0 + GRP):
                    n = len(ops[j])
                    for k, (t, src) in enumerate(ops[j]):
                        if t is None:
                            lhs = ident[:]
                        else:
                            dt = dtp.tile([128, 128], F16, tag="dt", name="dt")
                            nc.scalar.activation(dt[:], ident[:], ACT.Copy,
                                                 scale=W_tm[:, j:j + 1, t:t + 1])
                            lhs = dt[:]
                        nc.tensor.matmul(pss[j][:, 0:C], lhs, src,
                                         start=(k == 0), stop=(k == n - 1))
                    nc.scalar.copy(acc2[:, j, :], pss[j][:, 0:C])

        # sweep A: 6 even-e taps -> acc2 (runs alongside DVE phase 1)
        pe_sweep(SWEEP_A, fold_acc2=False, fold_accd=False)
        # B_p2/B_m2 copies: emitted here so their WAR waits (on sweep A's
        # PE-side A_p2/A_m2 reads) sit ahead of only sweep-B work
        B_p2 = vd_tile("vdg0", "B_p2"); shift_copy(nc.scalar, B_p2, 66)
        B_m2 = vd_tile("vdg1", "B_m2"); shift_copy(nc.scalar, B_m2, 62)
        BF.update({2: B_p2, -2: B_m2})
        # sweep B: 4 odd-e taps + acc2 -> acc2 (independent of acc_d)
        pe_sweep(SWEEP_B, fold_acc2=True, fold_accd=False)
        cmE.__exit__(None, None, None)
        # final merge on DVE right after its last tap (fp16 2x mode)
        for j in range(NCHUNK):
            nc.vector.tensor_tensor(acc_d[:, j, :], acc_d[:, j, :],
                                    acc2[:, j, :], op=ALU.add)
        if debug:
            nc.gpsimd.dma_start(
                d["dbg_acc"].rearrange("p (j c) -> p j c", j=NCHUNK), acc_d[:])

        # ---- phase G: xf transpose-evac + relu/maxpool chain ----
        # x1 = relu(relu(xfT) + m1^T); x2 = relu(x1 + m2^T)  (x2 in-place in m2;
        # x1 reuses the DVE vd slots, which are dead after phase E)
        cmG = tc.tile_pool(name="psG", bufs=4, space="PSUM"); psG = cmG.__enter__()
        x1 = [main.tile([128, L], F16, tag=f"vdd{cc}", name=f"x1_{cc}")
              for cc in range(2)]
        for j2 in range(NCHUNK // 2):
            for cc in range(2):
                pt = psG.tile([128, 2, 128], F16, tag="tp", name="tp")
                for u in range(2):
                    nc.tensor.transpose(
                        pt[:, u, :],
                        acc_d[:, 2 * j2 + u, 128 * cc:128 * (cc + 1)], ident[:])
                nc.scalar.activation(x1[cc][:, 256 * j2:256 * (j2 + 1)],
                                     pt.rearrange("p a b -> p (a b)"), ACT.Relu)
        cmG.__exit__(None, None, None)
        x2 = m2
        for cc in range(2):
            nc.vector.tensor_tensor(x1[cc][:], x1[cc][:], m1[cc][:], op=ALU.add)
            nc.scalar.activation(x1[cc][:], x1[cc][:], ACT.Relu)
            nc.vector.tensor_tensor(x2[cc][:], x1[cc][:], m2[cc][:], op=ALU.add)
            nc.scalar.activation(x2[cc][:], x2[cc][:], ACT.Relu)
        if debug:
            for cc in range(2):
                nc.gpsimd.dma_start(d["dbg_x1"][128 * cc:128 * (cc + 1), :], x1[cc][:])

        # ---- phase H: fu matmul + bias + relu + residual, incremental BN ----
        cmH = tc.tile_pool(name="psH", bufs=2, space="PSUM"); psH = cmH.__enter__()
        out_all = main.tile([128, 2, L], F16, tag="acc", name="out_all")
        out_cm = [out_all[:, cc, :] for cc in range(2)]
        st = small.tile([128, 2, 8, 6], F32, tag="bnst", name="bnst")
        rhss = [x1[0], x1[1], x2[0], x2[1]]
        for mc in range(2):
            for half in range(2):
                ps = psH.tile([128, 4, 512], F32, tag="fups", name="fups")
                for q in range(4):
                    n8 = 4 * half + q
                    for kc in range(4):
                        nc.tensor.matmul(ps[:, q, :], wfu_sb[:, kc, mc, :],
                                         rhss[kc][:, 512 * n8:512 * (n8 + 1)],
                                         start=(kc == 0), stop=False)
                    nc.tensor.matmul(ps[:, q, :],
                                     bfu_row[:, 128 * mc:128 * (mc + 1)],
                                     onesr[:], start=False, stop=True)
                for q in range(4):
                    n8 = 4 * half + q
                    sl = slice(512 * n8, 512 * (n8 + 1))
                    nc.scalar.activation(out_cm[mc][:, sl], ps[:, q, :], ACT.Relu)
                    nc.vector.tensor_tensor(out_cm[mc][:, sl], out_cm[mc][:, sl],
                                            xT_cm[mc][:, sl], op=ALU.add)
                    nc.vector.bn_stats(st[:, mc, n8, :], out_cm[mc][:, sl])
        cmH.__exit__(None, None, None)

        # ---- BN: pack local sums, single AllReduce, normalize ----
        bnpack = small.tile([128, 4], F32, tag="bnpack", name="bnpack")
        for mc in range(2):
            ag = small.tile([128, 2], F32, tag="bnag", name="bnag", bufs=2)
            nc.vector.bn_aggr(ag[:], st[:, mc])
            nc.vector.tensor_scalar(bnpack[:, 2 * mc:2 * mc + 1], ag[:, 0:1],
                                    float(L), None, op0=ALU.mult)
            sq = small.tile([128, 1], F32, tag="bnsq", name="bnsq", bufs=2)
            nc.vector.tensor_tensor(sq[:], ag[:, 0:1], ag[:, 0:1], op=ALU.mult)
            nc.vector.tensor_tensor(sq[:], sq[:], ag[:, 1:2], op=ALU.add)
            nc.vector.tensor_scalar(bnpack[:, 2 * mc + 1:2 * mc + 2], sq[:],
                                    float(L), None, op0=ALU.mult)
        cin = dram.tile([128, 4], F32, name="cin")
        cout = dram.tile([128, 4], F32, name="cout")
        nc.sync.dma_start(cin[:], bnpack[:])
        nc.gpsimd.collective_compute(
            "AllReduce", ALU.add, replica_groups=[list(range(n_cores))],
            ins=[cin.opt()], outs=[cout.opt()])
        gs = small.tile([128, 4], F32, tag="gs", name="gs")
        nc.sync.dma_start(gs[:], cout[:])

        NTOT = float(n_cores * L)
        scale = small.tile([128, 2], F32, tag="scale", name="scale")
        shift = small.tile([128, 2], F32, tag="shift", name="shift")
        mean = small.tile([128, 2], F32, tag="mean", name="mean")
        var = small.tile([128, 2], F32, tag="var", name="var")
        for cc in range(2):
            nc.vector.tensor_scalar(mean[:, cc:cc + 1], gs[:, 2 * cc:2 * cc + 1],
                                    1.0 / NTOT, None, op0=ALU.mult)
            nc.vector.tensor_scalar(var[:, cc:cc + 1], gs[:, 2 * cc + 1:2 * cc + 2],
                                    1.0 / NTOT, None, op0=ALU.mult)
        msq = small.tile([128, 2], F32, tag="msq", name="msq")
        nc.vector.tensor_tensor(msq[:], mean[:], mean[:], op=ALU.mult)
        nc.vector.tensor_tensor(var[:], var[:], msq[:], op=ALU.subtract)
        rs = small.tile([128, 2], F32, tag="rs", name="rs")
        nc.vector.tensor_scalar(var[:], var[:], float(EPS), None, op0=ALU.add)
        nc.scalar.activation(rs[:], var[:], ACT.Sqrt)
        nc.vector.reciprocal(rs[:], rs[:])
        nc.vector.tensor_tensor(scale[:], gamma2[:], rs[:], op=ALU.mult)
        nc.vector.tensor_tensor(shift[:], mean[:], scale[:], op=ALU.mult)
        nc.vector.tensor_tensor(shift[:], beta2[:], shift[:], op=ALU.subtract)

        # normalize in place (fp16 4x), DMA out channel-major fp16
        for cc in range(2):
            for hh in range(2):
                sl = slice(2048 * hh, 2048 * (hh + 1))
                nc.vector.tensor_scalar(out_cm[cc][:, sl], out_cm[cc][:, sl],
                                        scale[:, cc:cc + 1], shift[:, cc:cc + 1],
                                        op0=ALU.mult, op1=ALU.add)
                nc.sync.dma_start(d["y"][128 * cc:128 * (cc + 1), sl],
                                  out_cm[cc][:, sl])


_CACHE = {}


def _get_program(n_cores=N_CORES, debug=False):
    key = (n_cores, debug)
    if key not in _CACHE:
        nc = bacc.Bacc("TRN2", target_bir_lowering=False, debug=False,
                       num_devices=n_cores)
        build(nc, n_cores, debug)
        nc.compile()
        _CACHE[key] = nc
    return _CACHE[key]


def make_in_map(inputs, b):
    consts = host_consts()
    f16 = np.float16
    # host-side grid permutation: xf16[u*64+v, c] = x[v, u, c]
    xf16 = np.ascontiguousarray(
        np.asarray(inputs["x"][b]).transpose(1, 0, 2).reshape(L, C)).astype(f16)
    wv = np.ascontiguousarray(
        np.asarray(inputs["Wv"], np.float32).reshape(2, 128, C)
        .transpose(1, 0, 2).reshape(128, 2 * C)).astype(f16)
    wa = np.ascontiguousarray(
        np.asarray(inputs["Wa"], np.float32).reshape(2, 128, 81)
        .transpose(1, 0, 2).reshape(128, 2 * 81)).astype(f16)
    wfu = np.ascontiguousarray(
        np.asarray(inputs["Wfu"], np.float32).reshape(4, 128, 2, 128)
        .transpose(1, 0, 2, 3).reshape(128, 4 * 2 * 128)).astype(f16)
    return {
        "xf16": xf16,
        "wv": wv, "wa": wa, "wfu": wfu,
        "bvrow": np.asarray(inputs["bv"], np.float32).reshape(1, C).astype(f16),
        "barow": np.ascontiguousarray(
            np.asarray(inputs["ba"], np.float32).reshape(81, 1)),
        "bfurow": np.asarray(inputs["bfu"], np.float32).reshape(1, C).astype(f16),
        "gamma2": np.ascontiguousarray(
            np.asarray(inputs["gamma"], np.float32).reshape(2, 128).T),
        "beta2": np.ascontiguousarray(
            np.asarray(inputs["beta"], np.float32).reshape(2, 128).T),
        **consts,
    }


def gather_out(res_y):
    # y[c, u*64+v] -> out[u, v, c], cast fp16 -> f32 on host
    return np.asarray(res_y, dtype=np.float32).reshape(C, H, W).transpose(1, 2, 0)


def kernel(**inputs):
    nc = _get_program()
    in_maps = [make_in_map(inputs, b) for b in range(B)]
    res = run_bass_kernel_spmd(nc, in_maps, list(range(N_CORES)))
    out = np.stack([gather_out(res.results[b]["y"]) for b in range(B)])
    return out.astype(np.float32)


# revision 25
# speedup vs baseline: 1.3716x; 1.0829x over previous
"""Trainium2 Bass kernel for nn_MOA_13254269075617 (sparse windowed attention block).

Sharding: data-parallel over batch B=8 across 8 NeuronCores (1 image each).
BatchNorm uses global batch stats via one on-device AllReduce of per-channel
sum / sum-of-squares (plus an early warm-up collective to absorb CC-stream
startup cost).

Per-core pipeline (all in the spatially-TRANSPOSED frame T(z)[u,v]=z[v,u],
fp16 end-to-end so DVE elementwise ops run in 2x/4x perf modes):
  xT_cm  : x in channel-major [256, 4096], host-pre-permuted grid, loaded
           via 4 big transpose-DMAs
  vT     : (xT @ Wv + bv) token-major [128, 32, 256] fp16
  AE     : exp-logits pq-major [81, 4096] on a zero-padded 66-pitch grid
  W      : 25-tap position-varying stencil weights (fold+attention combined)
  vd     : 9 token-shifted copies of vT via SBUF->SBUF DMA (no PE work)
  acc    : 25-tap stencil apply, token-major fp16 FMAs on DVE (2x mode)
  x1/x2  : relu chains with 3x3/5x5 maxpools (separable shifted-max trees)
  out    : concat-matmul (Wfu) + residual, BN with AllReduce'd stats
  y      : channel-major fp16 output, host casts/transposes to f32 [H,W,C]
"""
import sys

for _p in (
    "/root/.axon_site",
    "/root/.axon_site/_ro/trn_rl_repo",
    "/root/.axon_site/_ro/pypackages",
    "/opt/trn_rl_repo",
):
    if _p not in sys.path:
        sys.path.append(_p)

from itertools import product

import numpy as np

import concourse.bass as bass
import concourse.tile as tile
from concourse import bacc, mybir
from concourse.bass_utils import run_bass_kernel_spmd

F32 = mybir.dt.float32
F16 = mybir.dt.float16
ALU = mybir.AluOpType
ACT = mybir.ActivationFunctionType

B, H, W, C = 8, 64, 64, 256
HOST_BN = True   # cross-core BN reduction on host (hint allows local stats;
                 # this is exact) -- removes both collectives + entry barrier
L = H * W                      # 4096 tokens
NCHUNK = L // 128              # 32 token chunks
N_CORES = 8
EPS = 1e-5

# (e, f) tap -> t index and source shift delta = 64*e + f
TAPI = {(e, f): (e + 2) * 5 + (f + 2) for e in range(-2, 3) for f in range(-2, 3)}


def host_consts():
    """Selector matrices and small constants (host-precomputed, same all cores)."""
    selsum = np.zeros((81, 9), np.float32)
    for p in range(9):
        selsum[9 * p:9 * p + 9, p] = 1.0
    selrep = np.zeros((9, 81), np.float32)
    for p in range(9):
        selrep[p, 9 * p:9 * p + 9] = 1.0
    # selshift[:, 25*d + tap]: for (di,dj) block d, tap (e,f):
    #   k = 9*(3di+dj) + 3(di+e)+(dj+f) if di+e,dj+f in [0,3)
    selshift = np.zeros((81, 9 * 25), np.float32)
    for d, (di, dj) in enumerate(product(range(3), range(3))):
        for t, (e, f) in enumerate(product(range(-2, 3), range(-2, 3))):
            dip, djp = di + e, dj + f
            if 0 <= dip < 3 and 0 <= djp < 3:
                k = 9 * (3 * di + dj) + (3 * dip + djp)
                selshift[k, 25 * d + t] = 1.0
    wmask = np.ones((25, 64, 64), np.float32)
    for t, (e, f) in enumerate(product(range(-2, 3), range(-2, 3))):
        if e > 0: wmask[t, 64 - e:, :] = 0
        if e < 0: wmask[t, :-e, :] = 0
        if f > 0: wmask[t, :, 64 - f:] = 0
        if f < 0: wmask[t, :, :-f] = 0
    f16 = np.float16
    return {
        "selsum": selsum.astype(f16),
        "selrep": selrep.astype(f16),
        "selshift": selshift.astype(f16),
        "wmask": wmask.reshape(25, 4096).astype(f16),
        "ident": np.eye(128, dtype=f16),
        "onesr": np.ones((1, 512), f16),
    }


def build(nc, n_cores, debug=False):
    d = {}
    def din(name, shape, dt=F16):
        d[name] = nc.dram_tensor(name, list(shape), dt, kind="ExternalInput").ap()
    def dout(name, shape, dt=F16):
        d[name] = nc.dram_tensor(name, list(shape), dt, kind="ExternalOutput").ap()

    din("xf16", (L, C))
    din("wv", (128, 2 * C)); din("bvrow", (1, C))
    din("wa", (128, 2 * 81)); din("barow", (81, 1), F32)
    din("wfu", (128, 4 * 2 * 128)); din("bfurow", (1, C))
    din("gamma2", (128, 2), F32); din("beta2", (128, 2), F32)
    din("selsum", (81, 9)); din("selrep", (9, 81)); din("selshift", (81, 225))
    din("ident", (128, 128)); din("onesr", (1, 512)); din("wmask", (25, L))
    dout("y", (2 * 128, L))
    dout("stats", (128, 4), F32)
    if debug:
        dout("dbg_vt", (128, NCHUNK * C))
        dout("dbg_w", (25, L))
        dout("dbg_acc", (128, NCHUNK * C))
        dout("dbg_x1", (2 * 128, L))

    with tile.TileContext(nc) as tc:
        _build_tc(tc, d, n_cores, debug)
    return d


def _build_tc(tc, d, n_cores, debug):
    nc = tc.nc
    from contextlib import ExitStack
    es = ExitStack()
    with es:
        consts = es.enter_context(tc.tile_pool(name="consts", bufs=1))
        main = es.enter_context(tc.tile_pool(name="main", bufs=1))
        small = es.enter_context(tc.tile_pool(name="small", bufs=1))
        dram = es.enter_context(tc.tile_pool(name="dram", bufs=2, space="DRAM"))

        # ---- phase A: input loads first (run under the CC entry barrier) ----
        xT_cm = [main.tile([128, L], F16, tag=f"xcm{cc}", name=f"xT_cm{cc}")
                 for cc in range(2)]
        for cc in range(2):
            for q in range(2):
                nc.sync.dma_start_transpose(
                    xT_cm[cc][:, 2048 * q:2048 * (q + 1)],
                    d["xf16"][2048 * q:2048 * (q + 1), 128 * cc:128 * (cc + 1)])

        # ---- const loads on the scalar HWDGE queue ----
        def cload(name, shape, dt=F16, src=None):
            t = consts.tile(list(shape), dt, tag=name, name=name)
            nc.scalar.dma_start(t[:], (src if src is not None else d[name])[:])
            return t
        ident = cload("ident", (128, 128))
        onesr = cload("onesr", (1, 512))
        bv_row = cload("bvrow", (1, C))
        bfu_row = cload("bfurow", (1, C))
        ba_sb = cload("barow", (81, 1), F32)
        selsum = cload("selsum", (81, 9))
        selrep = cload("selrep", (9, 81))
        selshift = cload("selshift", (81, 225))
        gamma2 = cload("gamma2", (128, 2), F32)
        beta2 = cload("beta2", (128, 2), F32)
        wv_sb = cload("wv", (128, 2, C), src=d["wv"].rearrange("p (k c) -> p k c", k=2))
        wa_sb = cload("wa", (128, 2, 81), src=d["wa"].rearrange("p (k c) -> p k c", k=2))
        wfu_sb = cload("wfu", (128, 4, 2, 128),
                       src=d["wfu"].rearrange("p (k m c) -> p k m c", k=4, m=2))

        if not HOST_BN:
            # warm-up collective: absorbs CC stream startup (~20us) early
            warm_in = dram.tile([1, 1], F32, name="warm_in")
            warm_out = dram.tile([1, 1], F32, name="warm_out")
            nc.sync.dma_start(warm_in[:], ba_sb[0:1, 0:1])
            nc.gpsimd.collective_compute(
                "AllReduce", ALU.add, replica_groups=[list(range(n_cores))],
                ins=[warm_in.opt()], outs=[warm_out.opt()])

        # ---- phase C: attention logits -> exp -> normalize ----
        # AE grid: (g1=u, g2=v); AE[g1+1, g2+1] = softmax-numerator of the
        # ORIGINAL position (h=g2, w=g1) (x transposed-grid ordering).
        cmMid = tc.tile_pool(name="mid", bufs=1); mid = cmMid.__enter__()
        wmask = mid.tile([25, L], F16, tag="wmask", name="wmask")
        nc.scalar.dma_start(wmask[:], d["wmask"][:])
        cmCl = tc.tile_pool(name="psClog", bufs=1, space="PSUM"); psCl = cmCl.__enter__()
        AE = mid.tile([81, 66 * 67], F16, tag="AE", name="AE")
        AE3 = AE.rearrange("p (r s) -> p r s", r=67)
        nc.vector.memset(AE3[:, 0:1, :], 0.0)          # pad ring only; the
        nc.vector.memset(AE3[:, 65:67, :], 0.0)        # 64x64 interior is
        nc.vector.memset(AE3[:, 1:65, 0:1], 0.0)       # fully written by exp
        nc.vector.memset(AE3[:, 1:65, 65:66], 0.0)

        for grp in range(2):
            pss = [psCl.tile([81, 512], F32, tag=f"aps{q}", name=f"aps{q}")
                   for q in range(4)]
            for kc in range(2):
                for q in range(4):
                    n8 = 4 * grp + q
                    nc.tensor.matmul(pss[q][:], wa_sb[:, kc, :],
                                     xT_cm[kc][:, 512 * n8:512 * (n8 + 1)],
                                     start=(kc == 0), stop=(kc == 1))
            for q in range(4):
                n8 = 4 * grp + q
                nc.scalar.activation(AE3[:, 1 + 8 * n8:1 + 8 * n8 + 8, 1:65],
                                     pss[q].rearrange("p (r s) -> p r s", s=64),
                                     ACT.Exp, bias=ba_sb[:, 0:1])

        # ---- phase B: vT = xT @ Wv + bv, token-major fp16 ----
        # (emitted between C-logits and C-rowsums so vT lands early for the
        # shifted-copy DMAs; PE queue order = emission order)
        vT = main.tile([128, NCHUNK, C], F16, tag="vT", name="vT")
        cmB = tc.tile_pool(name="psB", bufs=2, space="PSUM"); psB = cmB.__enter__()
        for g in range(8):
            ps = psB.tile([128, 4, C], F32, tag="bps", name="bps")
            for jj in range(4):
                j = 4 * g + jj
                for kc in range(2):
                    nc.tensor.matmul(ps[:, jj, :],
                                     xT_cm[kc][:, 128 * j:128 * (j + 1)],
                                     wv_sb[:, kc, :], start=(kc == 0), stop=False)
                nc.tensor.matmul(ps[:, jj, :], onesr[:, 0:128], bv_row[:],
                                 start=False, stop=True)
            for jj in range(4):
                nc.scalar.copy(vT[:, 4 * g + jj, :], ps[:, jj, :])
        cmB.__exit__(None, None, None)
        cmCl.__exit__(None, None, None)
        if debug:
            nc.gpsimd.dma_start(
                d["dbg_vt"].rearrange("p (j c) -> p j c", j=NCHUNK), vT[:])

        # ---- phase C (cont.): per-p row sums via selector matmuls ----
        cmC = tc.tile_pool(name="psCrow", bufs=2, space="PSUM"); psC = cmC.__enter__()
        ROWCH = [(r0, min(7, 64 - r0)) for r0 in range(0, 64, 7)]
        for r0, nr in ROWCH:
            N = nr * 66
            win = slice((r0 + 1) * 66, (r0 + 1) * 66 + N)
            ps = psC.tile([9, 512], F32, tag="sps", name="sps")
            nc.tensor.matmul(ps[:, 0:N], selsum[:], AE[:, win],
                             start=True, stop=True)
            rchf = small.tile([9, 512], F32, tag="rchf", name="rchf", bufs=2)
            nc.vector.reciprocal_approx_fast(rchf[:, 0:N], ps[:, 0:N])
            rch = small.tile([9, 512], F16, tag="rch", name="rch", bufs=2)
            nc.scalar.copy(rch[:, 0:N], rchf[:, 0:N])
            ps2 = psC.tile([81, 512], F32, tag="rps", name="rps")
            nc.tensor.matmul(ps2[:, 0:N], selrep[:], rch[:, 0:N],
                             start=True, stop=True)
            rt = small.tile([81, 512], F16, tag="rt", name="rt", bufs=2)
            nc.scalar.copy(rt[:, 0:N], ps2[:, 0:N])
            iv = AE3[:, r0 + 1:r0 + 1 + nr, 1:65]
            nc.vector.tensor_tensor(
                iv, iv, rt[:, 0:N].rearrange("p (r s) -> p r s", s=66)[:, :, 1:65],
                op=ALU.mult)
        cmC.__exit__(None, None, None)

        # ---- phase D: W stencil build (9 shifted selector matmuls) ----
        cmD = tc.tile_pool(name="psD", bufs=1, space="PSUM"); psD = cmD.__enter__()
        W_tap = mid.tile([25, L], F16, tag="wtap", name="W_tap")
        wmask_t = wmask.rearrange("p (u v) -> p v u", u=64)
        wtap_t = W_tap.rearrange("p (u v) -> p v u", u=64)
        for gstart in (0, 5):
            grp = ROWCH[gstart:gstart + 5]
            pss = [psD.tile([25, 512], F32, tag=f"wps{q}", name=f"wps{q}")
                   for q in range(len(grp))]
            for dd, (di, dj) in enumerate(product(range(3), range(3))):
                for q, (r0, nr) in enumerate(grp):
                    N = nr * 66
                    st = (r0 + 2 - dj) * 66 + (2 - di)
                    nc.tensor.matmul(pss[q][:, 0:N],
                                     selshift[:, 25 * dd:25 * (dd + 1)],
                                     AE[:, st:st + N],
                                     start=(dd == 0), stop=(dd == 8))
            for q, (r0, nr) in enumerate(grp):
                N = nr * 66
                wt = small.tile([25, 512], F16, tag="wt", name="wt", bufs=2)
                nc.scalar.copy(wt[:, 0:N], pss[q][:, 0:N])
                nc.vector.tensor_tensor(
                    wtap_t[:, r0:r0 + nr, :],
                    wt[:, 0:N].rearrange("p (r s) -> p r s", s=66)[:, :, 0:64],
                    wmask_t[:, r0:r0 + nr, :], op=ALU.mult)
        if debug:
            nc.gpsimd.dma_start(d["dbg_w"][:], W_tap[:])
        cmD.__exit__(None, None, None)
        cmD2 = tc.tile_pool(name="psD2", bufs=2, space="PSUM"); psD2 = cmD2.__enter__()
        W_tm = main.tile([128, NCHUNK, 25], F32, tag="W_tm", name="W_tm")
        for j in range(NCHUNK):
            pt = psD2.tile([128, 25], F16, tag="wtp", name="wtp")
            nc.tensor.transpose(pt[:], W_tap[:, 128 * j:128 * (j + 1)],
                                ident[0:25, 0:25])
            nc.scalar.copy(W_tm[:, j, :], pt[:])
        cmD2.__exit__(None, None, None)
        cmMid.__exit__(None, None, None)

        # ---- maxpools first in the DVE stream (only need xT_cm; they fill
        # the head while W/vd are still being built) ----
        ptmp = es.enter_context(tc.tile_pool(name="ptmp", bufs=1))
        m1 = [main.tile([128, L], F16, tag=f"m1{cc}", name=f"m1_{cc}") for cc in range(2)]
        m2 = [main.tile([128, L], F16, tag=f"m2{cc}", name=f"m2_{cc}") for cc in range(2)]

        def g3(ap):
            return ap.rearrange("p (h w) -> p h w", h=64)

        def hmax3(eng, dst, src):
            dv, sv = g3(dst), g3(src)
            t1 = ptmp.tile([128, L], F16, tag="ptmp", name="ptmp")
            tv = g3(t1)
            eng.tensor_tensor(tv[:, :, 1:], sv[:, :, 1:], sv[:, :, :63], op=ALU.max)
            nc.scalar.copy(tv[:, :, 0:1], sv[:, :, 0:1])
            eng.tensor_tensor(dv[:, :, :63], tv[:, :, :63], sv[:, :, 1:], op=ALU.max)
            nc.scalar.copy(dv[:, :, 63:64], tv[:, :, 63:64])

        def vmax3(eng, dst, src):
            dv, sv = g3(dst), g3(src)
            t1 = ptmp.tile([128, L], F16, tag="ptmp", name="ptmp")
            tv = g3(t1)
            eng.tensor_tensor(tv[:, 1:, :], sv[:, 1:, :], sv[:, :63, :], op=ALU.max)
            nc.scalar.copy(tv[:, 0:1, :], sv[:, 0:1, :])
            eng.tensor_tensor(dv[:, :63, :], tv[:, :63, :], sv[:, 1:, :], op=ALU.max)
            nc.scalar.copy(dv[:, 63:64, :], tv[:, 63:64, :])

        def hspread(eng, dst, src):
            dv, sv = g3(dst), g3(src)
            eng.tensor_tensor(dv[:, :, 1:63], sv[:, :, 0:62], sv[:, :, 2:64], op=ALU.max)
            nc.scalar.copy(dv[:, :, 0:1], sv[:, :, 1:2])
            nc.scalar.copy(dv[:, :, 63:64], sv[:, :, 62:63])

        def vspread(eng, dst, src):
            dv, sv = g3(dst), g3(src)
            eng.tensor_tensor(dv[:, 1:63, :], sv[:, 0:62, :], sv[:, 2:64, :], op=ALU.max)
            nc.scalar.copy(dv[:, 0:1, :], sv[:, 1:2, :])
            nc.scalar.copy(dv[:, 63:64, :], sv[:, 62:63, :])

        for cc in range(2):
            eng = nc.vector
            cm3 = ptmp.tile([128, L], F16, tag="ptmp2", name="ptmp2")
            hmax3(eng, cm3, xT_cm[cc])
            vmax3(eng, m1[cc], cm3)
            cm5 = ptmp.tile([128, L], F16, tag="ptmp3", name="ptmp3")
            hspread(eng, cm5, cm3)
            r35 = ptmp.tile([128, L], F16, tag="ptmp2", name="ptmp2")
            vmax3(eng, r35, cm5)
            vspread(eng, m2[cc], r35)

        # ---- shifted copies of vT via SBUF->SBUF DMA ----
        # S(delta)[p, j] = v[128j + p + delta]; lanes whose source would leave
        # [0, 4096) are clamp-filled with real (finite) data -- their taps have
        # W == 0 via wmask, so any finite value is safe (never NaN).
        def shift_copy(eng, dst, delta):
            dd = abs(delta)
            if delta > 0:
                eng.dma_start(dst[0:128 - dd, :, :], vT[dd:128, :, :])
                eng.dma_start(dst[128 - dd:128, 0:NCHUNK - 1, :],
                              vT[0:dd, 1:NCHUNK, :])
                eng.dma_start(dst[128 - dd:128, NCHUNK - 1, :],
                              vT[0:dd, NCHUNK - 1, :])          # clamp (W=0)
            else:
                eng.dma_start(dst[dd:128, :, :], vT[0:128 - dd, :, :])
                eng.dma_start(dst[0:dd, 1:NCHUNK, :],
                              vT[128 - dd:128, 0:NCHUNK - 1, :])
                eng.dma_start(dst[0:dd, 0, :], vT[128 - dd:128, 0, :])  # clamp

        def vd_tile(tag, nm):
            return main.tile([128, NCHUNK, C], F16, tag=tag, name=nm)

        # A-family first, then edges, then the DVE-side B-family. Tag-reuse
        # WAR waits are always satisfied by readers on OTHER queues.
        # (B_p2/B_m2 are emitted later, between the two PE sweeps.)
        A_p1 = vd_tile("vdd0", "A_p1"); shift_copy(nc.sync, A_p1, 1)
        A_p2 = vd_tile("vdg0", "A_p2"); shift_copy(nc.scalar, A_p2, 2)
        A_m1 = vd_tile("vdd1", "A_m1"); shift_copy(nc.sync, A_m1, -1)
        A_m2 = vd_tile("vdg1", "A_m2"); shift_copy(nc.scalar, A_m2, -2)

        # edge tiles for tap (e=-1, f) at chunk 0: edgeB[p, fi] = v[p - 64 + f]
        # (valid lanes p >= 64; lanes < 64+|f| have W=0, clamp-filled)
        EDGEF = (0, 1, -1, 2, -2)
        FI = {f: i for i, f in enumerate(EDGEF)}
        edgeB = main.tile([128, 5, C], F16, tag="edgeB", name="edgeB")
        for fi, f in enumerate(EDGEF):
            if f >= 0:
                nc.sync.dma_start(edgeB[64:128, fi, :], vT[f:64 + f, 0, :])
                nc.sync.dma_start(edgeB[0:64, fi, :], vT[0:64, 0, :])
            else:
                nc.sync.dma_start(edgeB[64 - f:128, fi, :], vT[0:64 + f, 0, :])
                nc.sync.dma_start(edgeB[0:64 - f, fi, :], vT[0:64 - f, 0, :])

        B_0 = vd_tile("vdd0", "B_0"); shift_copy(nc.sync, B_0, 64)
        B_p1 = vd_tile("vdd1", "B_p1"); shift_copy(nc.sync, B_p1, 65)
        B_m1 = vd_tile("vdd0", "B_m1"); shift_copy(nc.sync, B_m1, 63)
        BF = {0: B_0, 1: B_p1, -1: B_m1}

        # ---- phase E: 25-tap apply ----
        # 13 taps as DVE scalar_tensor_tensor FMAs into acc_d; 12 taps on
        # ScalarE+PE: ScalarE builds diag(w) tiles (reads only ident/W_tm),
        # PE accumulates psum[j] += diag(w) @ v_shifted into per-chunk PSUM
        # banks, folding acc_d in at the end.  GPSIMD is useless here -- it
        # contends with DVE's SBUF port and its AP-scalar ops run at ~4us.
        acc_d = main.tile([128, NCHUNK, C], F16, tag="acc", name="acc_d")
        acc2 = main.tile([128, NCHUNK, C], F16, tag="accg", name="acc2")
        VDT = {0: vT, 1: A_p1, -1: A_m1, 2: A_p2, -2: A_m2}

        def tap_sources(e, f):
            """Yield (j, src_ap) for tap (e, f)."""
            if e % 2 == 0:
                vdt, off = VDT[f], e // 2
                for j in range(NCHUNK):
                    jp = j + off
                    if 0 <= jp < NCHUNK:
                        yield j, vdt[:, jp, :]
            elif e == 1:
                bt = BF[f]
                for j in range(NCHUNK):
                    yield j, bt[:, j, :]
            else:
                bt = BF[f]
                yield 0, edgeB[:, FI[f], :]
                for j in range(1, NCHUNK):
                    yield j, bt[:, j - 1, :]

        def dve_fma(j, src, t, first=False):
            wap = W_tm[:, j:j + 1, t:t + 1]
            dst = acc_d[:, j, :]
            if first:
                nc.vector.tensor_scalar(dst, src, wap, None, op0=ALU.mult)
            else:
                nc.vector.scalar_tensor_tensor(dst, src, wap, dst,
                                               op0=ALU.mult, op1=ALU.add)

        # DVE phase 1 (chunk-inner; sources available early)
        for j in range(NCHUNK):
            dve_fma(j, vT[:, j, :], TAPI[(0, 0)], first=True)
        for e, f in ((2, 0), (-2, 0), (0, 1), (2, 1), (-2, 1),
                     (0, -1), (2, -1), (-2, -1)):
            t = TAPI[(e, f)]
            for j, src in tap_sources(e, f):
                dve_fma(j, src, t)
        # DVE phase 2 (tap-major; B-family sources arrive while phase 1 runs)
        for e, f in ((1, 0), (-1, 0), (1, 1), (-1, 1), (1, -1), (-1, -1)):
            t = TAPI[(e, f)]
            for j, src in tap_sources(e, f):
                dve_fma(j, src, t)

        # ScalarE+PE path
        dtp = es.enter_context(tc.tile_pool(name="dtp", bufs=4))
        cmE = tc.tile_pool(name="psE", bufs=1, space="PSUM"); psE = cmE.__enter__()
        GRP = 8
        SWEEP_A = [(0, 2), (2, 2), (-2, 2), (0, -2), (2, -2), (-2, -2)]
        SWEEP_B = [(1, 2), (-1, 2), (1, -2), (-1, -2)]

        def pe_sweep(taps, fold_acc2, fold_accd):
            for g0 in range(0, NCHUNK, GRP):
                ops = {j: [] for j in range(g0, g0 + GRP)}
                for e, f in taps:
                    t = TAPI[(e, f)]
                    for j, src in tap_sources(e, f):
                        if g0 <= j < g0 + GRP:
                            ops[j].append((t, src))
                for j in range(g0, g0 + GRP):
                    if fold_acc2:
                        ops[j].append((None, acc2[:, j, :]))
                    if fold_accd:
                        ops[j].append((None, acc_d[:, j, :]))
                pss = {j: psE.tile([128, 512], F32, tag=f"eps{j - g0}",
                                   name=f"eps{j - g0}")
                       for j in range(g0, g0 + GRP)}
                for j in range(g0, g0 + GRP):
                    n = len(ops[j])
                    for k, (t, src) in enumerate(ops[j]):
                        if t is None:
                            lhs = ident[:]
                        else:
                            dt = dtp.tile([128, 128], F16, tag="dt", name="dt")
                            nc.scalar.activation(dt[:], ident[:], ACT.Copy,
                                                 scale=W_tm[:, j:j + 1, t:t + 1])
                            lhs = dt[:]
                        nc.tensor.matmul(pss[j][:, 0:C], lhs, src,
                                         start=(k == 0), stop=(k == n - 1))
                    nc.scalar.copy(acc2[:, j, :], pss[j][:, 0:C])

        # sweep A: 6 even-e taps -> acc2 (runs alongside DVE phase 1)
        pe_sweep(SWEEP_A, fold_acc2=False, fold_accd=False)
        # B_p2/B_m2 copies: emitted here so their WAR waits (on sweep A's
        # PE-side A_p2/A_m2 reads) sit ahead of only sweep-B work
        B_p2 = vd_tile("vdg0", "B_p2"); shift_copy(nc.scalar, B_p2, 66)
        B_m2 = vd_tile("vdg1", "B_m2"); shift_copy(nc.scalar, B_m2, 62)
        BF.update({2: B_p2, -2: B_m2})
        # sweep B: 4 odd-e taps + acc2 -> acc2 (independent of acc_d)
        pe_sweep(SWEEP_B, fold_acc2=True, fold_accd=False)
        cmE.__exit__(None, None, None)
        # final merge on DVE right after its last tap (fp16 2x mode)
        for j in range(NCHUNK):
            nc.vector.tensor_tensor(acc_d[:, j, :], acc_d[:, j, :],
                                    acc2[:, j, :], op=ALU.add)
        if debug:
            nc.gpsimd.dma_start(
                d["dbg_acc"].rearrange("p (j c) -> p j c", j=NCHUNK), acc_d[:])

        # ---- phase G: xf transpose-evac + relu/maxpool chain ----
        # x1 = relu(relu(xfT) + m1^T); x2 = relu(x1 + m2^T)  (x2 in-place in m2;
        # x1 reuses the DVE vd slots, which are dead after phase E)
        cmG = tc.tile_pool(name="psG", bufs=4, space="PSUM"); psG = cmG.__enter__()
        x1 = [main.tile([128, L], F16, tag=f"vdd{cc}", name=f"x1_{cc}")
              for cc in range(2)]
        for j2 in range(NCHUNK // 2):
            for cc in range(2):
                pt = psG.tile([128, 2, 128], F16, tag="tp", name="tp")
                for u in range(2):
                    nc.tensor.transpose(
                        pt[:, u, :],
                        acc_d[:, 2 * j2 + u, 128 * cc:128 * (cc + 1)], ident[:])
                nc.scalar.activation(x1[cc][:, 256 * j2:256 * (j2 + 1)],
                                     pt.rearrange("p a b -> p (a b)"), ACT.Relu)
        cmG.__exit__(None, None, None)
        x2 = m2
        for cc in range(2):
            nc.vector.tensor_tensor(x1[cc][:], x1[cc][:], m1[cc][:], op=ALU.add)
            nc.scalar.activation(x1[cc][:], x1[cc][:], ACT.Relu)
            nc.vector.tensor_tensor(x2[cc][:], x1[cc][:], m2[cc][:], op=ALU.add)
            nc.scalar.activation(x2[cc][:], x2[cc][:], ACT.Relu)
        if debug:
            for cc in range(2):
                nc.gpsimd.dma_start(d["dbg_x1"][128 * cc:128 * (cc + 1), :], x1[cc][:])

        # ---- phase H: fu matmul + bias + relu + residual, incremental BN ----
        cmH = tc.tile_pool(name="psH", bufs=2, space="PSUM"); psH = cmH.__enter__()
        out_all = main.tile([128, 2, L], F16, tag="acc", name="out_all")
        out_cm = [out_all[:, cc, :] for cc in range(2)]
        st = small.tile([128, 2, 8, 6], F32, tag="bnst", name="bnst")
        rhss = [x1[0], x1[1], x2[0], x2[1]]
        for mc in range(2):
            for half in range(2):
                ps = psH.tile([128, 4, 512], F32, tag="fups", name="fups")
                for q in range(4):
                    n8 = 4 * half + q
                    for kc in range(4):
                        nc.tensor.matmul(ps[:, q, :], wfu_sb[:, kc, mc, :],
                                         rhss[kc][:, 512 * n8:512 * (n8 + 1)],
                                         start=(kc == 0), stop=False)
                    nc.tensor.matmul(ps[:, q, :],
                                     bfu_row[:, 128 * mc:128 * (mc + 1)],
                                     onesr[:], start=False, stop=True)
                for q in range(4):
                    n8 = 4 * half + q
                    sl = slice(512 * n8, 512 * (n8 + 1))
                    nc.scalar.activation(out_cm[mc][:, sl], ps[:, q, :], ACT.Relu)
                    nc.vector.tensor_tensor(out_cm[mc][:, sl], out_cm[mc][:, sl],
                                            xT_cm[mc][:, sl], op=ALU.add)
                    nc.vector.bn_stats(st[:, mc, n8, :], out_cm[mc][:, sl])
        cmH.__exit__(None, None, None)

        # ---- BN: pack local sums, single AllReduce, normalize ----
        bnpack = small.tile([128, 4], F32, tag="bnpack", name="bnpack")
        for mc in range(2):
            ag = small.tile([128, 2], F32, tag="bnag", name="bnag", bufs=2)
            nc.vector.bn_aggr(ag[:], st[:, mc])
            nc.vector.tensor_scalar(bnpack[:, 2 * mc:2 * mc + 1], ag[:, 0:1],
                                    float(L), None, op0=ALU.mult)
            sq = small.tile([128, 1], F32, tag="bnsq", name="bnsq", bufs=2)
            nc.vector.tensor_tensor(sq[:], ag[:, 0:1], ag[:, 0:1], op=ALU.mult)
            nc.vector.tensor_tensor(sq[:], sq[:], ag[:, 1:2], op=ALU.add)
            nc.vector.tensor_scalar(bnpack[:, 2 * mc + 1:2 * mc + 2], sq[:],
                                    float(L), None, op0=ALU.mult)
        if HOST_BN:
            # ship local sums + unnormalized activations; host finishes BN
            nc.sync.dma_start(d["stats"][:], bnpack[:])
            for cc in range(2):
                for hh in range(2):
                    sl = slice(2048 * hh, 2048 * (hh + 1))
                    nc.sync.dma_start(d["y"][128 * cc:128 * (cc + 1), sl],
                                      out_cm[cc][:, sl])
        else:
            cin = dram.tile([128, 4], F32, name="cin")
            cout = dram.tile([128, 4], F32, name="cout")
            nc.sync.dma_start(cin[:], bnpack[:])
            nc.gpsimd.collective_compute(
                "AllReduce", ALU.add, replica_groups=[list(range(n_cores))],
                ins=[cin.opt()], outs=[cout.opt()])
            gs = small.tile([128, 4], F32, tag="gs", name="gs")
            nc.sync.dma_start(gs[:], cout[:])

            NTOT = float(n_cores * L)
            scale = small.tile([128, 2], F32, tag="scale", name="scale")
            shift = small.tile([128, 2], F32, tag="shift", name="shift")
            mean = small.tile([128, 2], F32, tag="mean", name="mean")
            var = small.tile([128, 2], F32, tag="var", name="var")
            for cc in range(2):
                nc.vector.tensor_scalar(mean[:, cc:cc + 1], gs[:, 2 * cc:2 * cc + 1],
                                        1.0 / NTOT, None, op0=ALU.mult)
                nc.vector.tensor_scalar(var[:, cc:cc + 1], gs[:, 2 * cc + 1:2 * cc + 2],
                                        1.0 / NTOT, None, op0=ALU.mult)
            msq = small.tile([128, 2], F32, tag="msq", name="msq")
            nc.vector.tensor_tensor(msq[:], mean[:], mean[:], op=ALU.mult)
            nc.vector.tensor_tensor(var[:], var[:], msq[:], op=ALU.subtract)
            rs = small.tile([128, 2], F32, tag="rs", name="rs")
            nc.vector.tensor_scalar(var[:], var[:], float(EPS), None, op0=ALU.add)
            nc.scalar.activation(rs[:], var[:], ACT.Sqrt)
            nc.vector.reciprocal(rs[:], rs[:])
            nc.vector.tensor_tensor(scale[:], gamma2[:], rs[:], op=ALU.mult)
            nc.vector.tensor_tensor(shift[:], mean[:], scale[:], op=ALU.mult)
            nc.vector.tensor_tensor(shift[:], beta2[:], shift[:], op=ALU.subtract)

            for cc in range(2):
                for hh in range(2):
                    sl = slice(2048 * hh, 2048 * (hh + 1))
                    nc.vector.tensor_scalar(out_cm[cc][:, sl], out_cm[cc][:, sl],
                                            scale[:, cc:cc + 1], shift[:, cc:cc + 1],
                                            op0=ALU.mult, op1=ALU.add)
                    nc.sync.dma_start(d["y"][128 * cc:128 * (cc + 1), sl],
                                      out_cm[cc][:, sl])


_CACHE = {}


def _get_program(n_cores=N_CORES, debug=False):
    key = (n_cores, debug)
    if key not in _CACHE:
        nc = bacc.Bacc("TRN2", target_bir_lowering=False, debug=False,
                       num_devices=n_cores)
        build(nc, n_cores, debug)
        nc.compile()
        _CACHE[key] = nc
    return _CACHE[key]


def make_in_map(inputs, b):
    consts = host_consts()
    f16 = np.float16
    # host-side grid permutation: xf16[u*64+v, c] = x[v, u, c]
    xf16 = np.ascontiguousarray(
        np.asarray(inputs["x"][b]).transpose(1, 0, 2).reshape(L, C)).astype(f16)
    wv = np.ascontiguousarray(
        np.asarray(inputs["Wv"], np.float32).reshape(2, 128, C)
        .transpose(1, 0, 2).reshape(128, 2 * C)).astype(f16)
    wa = np.ascontiguousarray(
        np.asarray(inputs["Wa"], np.float32).reshape(2, 128, 81)
        .transpose(1, 0, 2).reshape(128, 2 * 81)).astype(f16)
    wfu = np.ascontiguousarray(
        np.asarray(inputs["Wfu"], np.float32).reshape(4, 128, 2, 128)
        .transpose(1, 0, 2, 3).reshape(128, 4 * 2 * 128)).astype(f16)
    return {
        "xf16": xf16,
        "wv": wv, "wa": wa, "wfu": wfu,
        "bvrow": np.asarray(inputs["bv"], np.float32).reshape(1, C).astype(f16),
        "barow": np.ascontiguousarray(
            np.asarray(inputs["ba"], np.float32).reshape(81, 1)),
        "bfurow": np.asarray(inputs["bfu"], np.float32).reshape(1, C).astype(f16),
        "gamma2": np.ascontiguousarray(
            np.asarray(inputs["gamma"], np.float32).reshape(2, 128).T),
        "beta2": np.ascontiguousarray(
            np.asarray(inputs["beta"], np.float32).reshape(2, 128).T),
        **consts,
    }


def gather_full(results, inputs):
    # y[c, u*64+v] -> out[u, v, c], cast fp16 -> f32 on host; with HOST_BN the
    # global batch-norm (exact, all 8 cores' stats) is applied here.
    ys = np.stack([np.asarray(results[b]["y"], dtype=np.float32)
                   for b in range(B)])                      # [B, C, L]
    if HOST_BN:
        st = np.stack([np.asarray(results[b]["stats"], dtype=np.float64)
                       for b in range(B)])                  # [B, 128, 4]
        st = st.sum(axis=0)
        s_pack = st.reshape(128, 2, 2)                      # [p, cc, (sum, sumsq)]
        cnt = float(B * L)
        mean = (s_pack[:, :, 0].T.reshape(C) / cnt)         # [C] (cc-major)
        ex2 = (s_pack[:, :, 1].T.reshape(C) / cnt)
        var = ex2 - mean * mean
        gamma = np.asarray(inputs["gamma"], np.float64)
        beta = np.asarray(inputs["beta"], np.float64)
        scale = gamma / np.sqrt(var + EPS)
        shift = beta - mean * scale
        ys = ys * scale[None, :, None] + shift[None, :, None]
    out = ys.reshape(B, C, H, W).transpose(0, 2, 3, 1)
    return np.ascontiguousarray(out, dtype=np.float32)


def kernel(**inputs):
    nc = _get_program()
    in_maps = [make_in_map(inputs, b) for b in range(B)]
    res = run_bass_kernel_spmd(nc, in_maps, list(range(N_CORES)))
    return gather_full(res.results, inputs)


# revision 26
# speedup vs baseline: 1.3740x; 1.0018x over previous
"""Trainium2 Bass kernel for nn_MOA_13254269075617 (sparse windowed attention block).

Sharding: data-parallel over batch B=8 across 8 NeuronCores (1 image each).
BatchNorm uses global batch stats via one on-device AllReduce of per-channel
sum / sum-of-squares (plus an early warm-up collective to absorb CC-stream
startup cost).

Per-core pipeline (all in the spatially-TRANSPOSED frame T(z)[u,v]=z[v,u],
fp16 end-to-end so DVE elementwise ops run in 2x/4x perf modes):
  xT_cm  : x in channel-major [256, 4096], host-pre-permuted grid, loaded
           via 4 big transpose-DMAs
  vT     : (xT @ Wv + bv) token-major [128, 32, 256] fp16
  AE     : exp-logits pq-major [81, 4096] on a zero-padded 66-pitch grid
  W      : 25-tap position-varying stencil weights (fold+attention combined)
  vd     : 9 token-shifted copies of vT via SBUF->SBUF DMA (no PE work)
  acc    : 25-tap stencil apply, token-major fp16 FMAs on DVE (2x mode)
  x1/x2  : relu chains with 3x3/5x5 maxpools (separable shifted-max trees)
  out    : concat-matmul (Wfu) + residual, BN with AllReduce'd stats
  y      : channel-major fp16 output, host casts/transposes to f32 [H,W,C]
"""
import sys

for _p in (
    "/root/.axon_site",
    "/root/.axon_site/_ro/trn_rl_repo",
    "/root/.axon_site/_ro/pypackages",
    "/opt/trn_rl_repo",
):
    if _p not in sys.path:
        sys.path.append(_p)

from itertools import product

import numpy as np

import concourse.bass as bass
import concourse.tile as tile
from concourse import bacc, mybir
from concourse.bass_utils import run_bass_kernel_spmd

F32 = mybir.dt.float32
F16 = mybir.dt.float16
ALU = mybir.AluOpType
ACT = mybir.ActivationFunctionType

B, H, W, C = 8, 64, 64, 256
HOST_BN = True   # cross-core BN reduction on host (hint allows local stats;
                 # this is exact) -- removes both collectives + entry barrier
L = H * W                      # 4096 tokens
NCHUNK = L // 128              # 32 token chunks
N_CORES = 8
EPS = 1e-5

# (e, f) tap -> t index and source shift delta = 64*e + f
TAPI = {(e, f): (e + 2) * 5 + (f + 2) for e in range(-2, 3) for f in range(-2, 3)}


def host_consts():
    """Selector matrices and small constants (host-precomputed, same all cores)."""
    selsum = np.zeros((81, 9), np.float32)
    for p in range(9):
        selsum[9 * p:9 * p + 9, p] = 1.0
    selrep = np.zeros((9, 81), np.float32)
    for p in range(9):
        selrep[p, 9 * p:9 * p + 9] = 1.0
    # selshift[:, 25*d + tap]: for (di,dj) block d, tap (e,f):
    #   k = 9*(3di+dj) + 3(di+e)+(dj+f) if di+e,dj+f in [0,3)
    selshift = np.zeros((81, 9 * 25), np.float32)
    for d, (di, dj) in enumerate(product(range(3), range(3))):
        for t, (e, f) in enumerate(product(range(-2, 3), range(-2, 3))):
            dip, djp = di + e, dj + f
            if 0 <= dip < 3 and 0 <= djp < 3:
                k = 9 * (3 * di + dj) + (3 * dip + djp)
                selshift[k, 25 * d + t] = 1.0
    wmask = np.ones((25, 64, 64), np.float32)
    for t, (e, f) in enumerate(product(range(-2, 3), range(-2, 3))):
        if e > 0: wmask[t, 64 - e:, :] = 0
        if e < 0: wmask[t, :-e, :] = 0
        if f > 0: wmask[t, :, 64 - f:] = 0
        if f < 0: wmask[t, :, :-f] = 0
    f16 = np.float16
    return {
        "selsum": selsum.astype(f16),
        "selrep": selrep.astype(f16),
        "selshift": selshift.astype(f16),
        "wmask": wmask.reshape(25, 4096).astype(f16),
        "ident": np.eye(128, dtype=f16),
        "onesr": np.ones((1, 512), f16),
    }


def build(nc, n_cores, debug=False):
    d = {}
    def din(name, shape, dt=F16):
        d[name] = nc.dram_tensor(name, list(shape), dt, kind="ExternalInput").ap()
    def dout(name, shape, dt=F16):
        d[name] = nc.dram_tensor(name, list(shape), dt, kind="ExternalOutput").ap()

    din("xf16", (L, C))
    din("wv", (128, 2 * C)); din("bvrow", (1, C))
    din("wa", (128, 2 * 81)); din("barow", (81, 1), F32)
    din("wfu", (128, 4 * 2 * 128)); din("bfurow", (1, C))
    din("gamma2", (128, 2), F32); din("beta2", (128, 2), F32)
    din("selsum", (81, 9)); din("selrep", (9, 81)); din("selshift", (81, 225))
    din("ident", (128, 128)); din("onesr", (1, 512)); din("wmask", (25, L))
    dout("y", (2 * 128, L))
    dout("stats", (128, 4), F32)
    if debug:
        dout("dbg_vt", (128, NCHUNK * C))
        dout("dbg_w", (25, L))
        dout("dbg_acc", (128, NCHUNK * C))
        dout("dbg_x1", (2 * 128, L))

    with tile.TileContext(nc) as tc:
        _build_tc(tc, d, n_cores, debug)
    return d


def _build_tc(tc, d, n_cores, debug):
    nc = tc.nc
    from contextlib import ExitStack
    es = ExitStack()
    with es:
        consts = es.enter_context(tc.tile_pool(name="consts", bufs=1))
        main = es.enter_context(tc.tile_pool(name="main", bufs=1))
        small = es.enter_context(tc.tile_pool(name="small", bufs=1))
        dram = es.enter_context(tc.tile_pool(name="dram", bufs=2, space="DRAM"))

        # ---- phase A: input loads first (run under the CC entry barrier) ----
        xT_cm = [main.tile([128, L], F16, tag=f"xcm{cc}", name=f"xT_cm{cc}")
                 for cc in range(2)]
        for cc in range(2):
            for q in range(2):
                nc.sync.dma_start_transpose(
                    xT_cm[cc][:, 2048 * q:2048 * (q + 1)],
                    d["xf16"][2048 * q:2048 * (q + 1), 128 * cc:128 * (cc + 1)])

        # ---- const loads on the scalar HWDGE queue ----
        def cload(name, shape, dt=F16, src=None):
            t = consts.tile(list(shape), dt, tag=name, name=name)
            nc.scalar.dma_start(t[:], (src if src is not None else d[name])[:])
            return t
        ident = cload("ident", (128, 128))
        onesr = cload("onesr", (1, 512))
        bv_row = cload("bvrow", (1, C))
        bfu_row = cload("bfurow", (1, C))
        ba_sb = cload("barow", (81, 1), F32)
        selsum = cload("selsum", (81, 9))
        selrep = cload("selrep", (9, 81))
        selshift = cload("selshift", (81, 225))
        gamma2 = cload("gamma2", (128, 2), F32)
        beta2 = cload("beta2", (128, 2), F32)
        wv_sb = cload("wv", (128, 2, C), src=d["wv"].rearrange("p (k c) -> p k c", k=2))
        wa_sb = cload("wa", (128, 2, 81), src=d["wa"].rearrange("p (k c) -> p k c", k=2))
        wfu_sb = cload("wfu", (128, 4, 2, 128),
                       src=d["wfu"].rearrange("p (k m c) -> p k m c", k=4, m=2))

        if not HOST_BN:
            # warm-up collective: absorbs CC stream startup (~20us) early
            warm_in = dram.tile([1, 1], F32, name="warm_in")
            warm_out = dram.tile([1, 1], F32, name="warm_out")
            nc.sync.dma_start(warm_in[:], ba_sb[0:1, 0:1])
            nc.gpsimd.collective_compute(
                "AllReduce", ALU.add, replica_groups=[list(range(n_cores))],
                ins=[warm_in.opt()], outs=[warm_out.opt()])

        # ---- phase C: attention logits -> exp -> normalize ----
        # AE grid: (g1=u, g2=v); AE[g1+1, g2+1] = softmax-numerator of the
        # ORIGINAL position (h=g2, w=g1) (x transposed-grid ordering).
        cmMid = tc.tile_pool(name="mid", bufs=1); mid = cmMid.__enter__()
        wmask = mid.tile([25, L], F16, tag="wmask", name="wmask")
        nc.scalar.dma_start(wmask[:], d["wmask"][:])
        cmCl = tc.tile_pool(name="psClog", bufs=1, space="PSUM"); psCl = cmCl.__enter__()
        AE = mid.tile([81, 66 * 67], F16, tag="AE", name="AE")
        AE3 = AE.rearrange("p (r s) -> p r s", r=67)
        nc.vector.memset(AE3[:, 0:1, :], 0.0)          # pad ring only; the
        nc.vector.memset(AE3[:, 65:67, :], 0.0)        # 64x64 interior is
        nc.vector.memset(AE3[:, 1:65, 0:1], 0.0)       # fully written by exp
        nc.vector.memset(AE3[:, 1:65, 65:66], 0.0)

        for grp in range(2):
            pss = [psCl.tile([81, 512], F32, tag=f"aps{q}", name=f"aps{q}")
                   for q in range(4)]
            for kc in range(2):
                for q in range(4):
                    n8 = 4 * grp + q
                    nc.tensor.matmul(pss[q][:], wa_sb[:, kc, :],
                                     xT_cm[kc][:, 512 * n8:512 * (n8 + 1)],
                                     start=(kc == 0), stop=(kc == 1))
            for q in range(4):
                n8 = 4 * grp + q
                nc.scalar.activation(AE3[:, 1 + 8 * n8:1 + 8 * n8 + 8, 1:65],
                                     pss[q].rearrange("p (r s) -> p r s", s=64),
                                     ACT.Exp, bias=ba_sb[:, 0:1])

        # ---- phase B: vT = xT @ Wv + bv, token-major fp16 ----
        # (emitted between C-logits and C-rowsums so vT lands early for the
        # shifted-copy DMAs; PE queue order = emission order)
        vT = main.tile([128, NCHUNK, C], F16, tag="vT", name="vT")
        cmB = tc.tile_pool(name="psB", bufs=2, space="PSUM"); psB = cmB.__enter__()
        for g in range(8):
            ps = psB.tile([128, 4, C], F32, tag="bps", name="bps")
            for jj in range(4):
                j = 4 * g + jj
                for kc in range(2):
                    nc.tensor.matmul(ps[:, jj, :],
                                     xT_cm[kc][:, 128 * j:128 * (j + 1)],
                                     wv_sb[:, kc, :], start=(kc == 0), stop=False)
                nc.tensor.matmul(ps[:, jj, :], onesr[:, 0:128], bv_row[:],
                                 start=False, stop=True)
            for jj in range(4):
                nc.scalar.copy(vT[:, 4 * g + jj, :], ps[:, jj, :])
        cmB.__exit__(None, None, None)
        cmCl.__exit__(None, None, None)
        if debug:
            nc.gpsimd.dma_start(
                d["dbg_vt"].rearrange("p (j c) -> p j c", j=NCHUNK), vT[:])

        # ---- phase C (cont.): per-p row sums via selector matmuls ----
        cmC = tc.tile_pool(name="psCrow", bufs=2, space="PSUM"); psC = cmC.__enter__()
        ROWCH = [(r0, min(7, 64 - r0)) for r0 in range(0, 64, 7)]
        for r0, nr in ROWCH:
            N = nr * 66
            win = slice((r0 + 1) * 66, (r0 + 1) * 66 + N)
            ps = psC.tile([9, 512], F32, tag="sps", name="sps")
            nc.tensor.matmul(ps[:, 0:N], selsum[:], AE[:, win],
                             start=True, stop=True)
            rchf = small.tile([9, 512], F32, tag="rchf", name="rchf", bufs=2)
            nc.vector.reciprocal_approx_fast(rchf[:, 0:N], ps[:, 0:N])
            rch = small.tile([9, 512], F16, tag="rch", name="rch", bufs=2)
            nc.scalar.copy(rch[:, 0:N], rchf[:, 0:N])
            ps2 = psC.tile([81, 512], F32, tag="rps", name="rps")
            nc.tensor.matmul(ps2[:, 0:N], selrep[:], rch[:, 0:N],
                             start=True, stop=True)
            rt = small.tile([81, 512], F16, tag="rt", name="rt", bufs=2)
            nc.scalar.copy(rt[:, 0:N], ps2[:, 0:N])
            iv = AE3[:, r0 + 1:r0 + 1 + nr, 1:65]
            nc.vector.tensor_tensor(
                iv, iv, rt[:, 0:N].rearrange("p (r s) -> p r s", s=66)[:, :, 1:65],
                op=ALU.mult)
        cmC.__exit__(None, None, None)

        # ---- phase D: W stencil build (9 shifted selector matmuls) ----
        cmD = tc.tile_pool(name="psD", bufs=1, space="PSUM"); psD = cmD.__enter__()
        W_tap = mid.tile([25, L], F16, tag="wtap", name="W_tap")
        wmask_t = wmask.rearrange("p (u v) -> p v u", u=64)
        wtap_t = W_tap.rearrange("p (u v) -> p v u", u=64)
        for gstart in (0, 5):
            grp = ROWCH[gstart:gstart + 5]
            pss = [psD.tile([25, 512], F32, tag=f"wps{q}", name=f"wps{q}")
                   for q in range(len(grp))]
            for dd, (di, dj) in enumerate(product(range(3), range(3))):
                for q, (r0, nr) in enumerate(grp):
                    N = nr * 66
                    st = (r0 + 2 - dj) * 66 + (2 - di)
                    nc.tensor.matmul(pss[q][:, 0:N],
                                     selshift[:, 25 * dd:25 * (dd + 1)],
                                     AE[:, st:st + N],
                                     start=(dd == 0), stop=(dd == 8))
            for q, (r0, nr) in enumerate(grp):
                N = nr * 66
                wt = small.tile([25, 512], F16, tag="wt", name="wt", bufs=2)
                nc.scalar.copy(wt[:, 0:N], pss[q][:, 0:N])
                nc.vector.tensor_tensor(
                    wtap_t[:, r0:r0 + nr, :],
                    wt[:, 0:N].rearrange("p (r s) -> p r s", s=66)[:, :, 0:64],
                    wmask_t[:, r0:r0 + nr, :], op=ALU.mult)
        if debug:
            nc.gpsimd.dma_start(d["dbg_w"][:], W_tap[:])
        cmD.__exit__(None, None, None)
        cmD2 = tc.tile_pool(name="psD2", bufs=2, space="PSUM"); psD2 = cmD2.__enter__()
        W_tm = main.tile([128, NCHUNK, 25], F32, tag="W_tm", name="W_tm")
        for j in range(NCHUNK):
            pt = psD2.tile([128, 25], F16, tag="wtp", name="wtp")
            nc.tensor.transpose(pt[:], W_tap[:, 128 * j:128 * (j + 1)],
                                ident[0:25, 0:25])
            nc.scalar.copy(W_tm[:, j, :], pt[:])
        cmD2.__exit__(None, None, None)
        cmMid.__exit__(None, None, None)

        # ---- maxpools first in the DVE stream (only need xT_cm; they fill
        # the head while W/vd are still being built) ----
        ptmp = es.enter_context(tc.tile_pool(name="ptmp", bufs=1))
        m1 = [main.tile([128, L], F16, tag=f"m1{cc}", name=f"m1_{cc}") for cc in range(2)]
        m2 = [main.tile([128, L], F16, tag=f"m2{cc}", name=f"m2_{cc}") for cc in range(2)]

        def g3(ap):
            return ap.rearrange("p (h w) -> p h w", h=64)

        def hmax3(eng, dst, src):
            dv, sv = g3(dst), g3(src)
            t1 = ptmp.tile([128, L], F16, tag="ptmp", name="ptmp")
            tv = g3(t1)
            eng.tensor_tensor(tv[:, :, 1:], sv[:, :, 1:], sv[:, :, :63], op=ALU.max)
            nc.scalar.copy(tv[:, :, 0:1], sv[:, :, 0:1])
            eng.tensor_tensor(dv[:, :, :63], tv[:, :, :63], sv[:, :, 1:], op=ALU.max)
            nc.scalar.copy(dv[:, :, 63:64], tv[:, :, 63:64])

        def vmax3(eng, dst, src):
            dv, sv = g3(dst), g3(src)
            t1 = ptmp.tile([128, L], F16, tag="ptmp", name="ptmp")
            tv = g3(t1)
            eng.tensor_tensor(tv[:, 1:, :], sv[:, 1:, :], sv[:, :63, :], op=ALU.max)
            nc.scalar.copy(tv[:, 0:1, :], sv[:, 0:1, :])
            eng.tensor_tensor(dv[:, :63, :], tv[:, :63, :], sv[:, 1:, :], op=ALU.max)
            nc.scalar.copy(dv[:, 63:64, :], tv[:, 63:64, :])

        def hspread(eng, dst, src):
            dv, sv = g3(dst), g3(src)
            eng.tensor_tensor(dv[:, :, 1:63], sv[:, :, 0:62], sv[:, :, 2:64], op=ALU.max)
            nc.scalar.copy(dv[:, :, 0:1], sv[:, :, 1:2])
            nc.scalar.copy(dv[:, :, 63:64], sv[:, :, 62:63])

        def vspread(eng, dst, src):
            dv, sv = g3(dst), g3(src)
            eng.tensor_tensor(dv[:, 1:63, :], sv[:, 0:62, :], sv[:, 2:64, :], op=ALU.max)
            nc.scalar.copy(dv[:, 0:1, :], sv[:, 1:2, :])
            nc.scalar.copy(dv[:, 63:64, :], sv[:, 62:63, :])

        for cc in range(2):
            eng = nc.vector
            cm3 = ptmp.tile([128, L], F16, tag="ptmp2", name="ptmp2")
            hmax3(eng, cm3, xT_cm[cc])
            vmax3(eng, m1[cc], cm3)
            cm5 = ptmp.tile([128, L], F16, tag="ptmp3", name="ptmp3")
            hspread(eng, cm5, cm3)
            r35 = ptmp.tile([128, L], F16, tag="ptmp2", name="ptmp2")
            vmax3(eng, r35, cm5)
            vspread(eng, m2[cc], r35)

        # ---- shifted copies of vT via SBUF->SBUF DMA ----
        # S(delta)[p, j] = v[128j + p + delta]; lanes whose source would leave
        # [0, 4096) are clamp-filled with real (finite) data -- their taps have
        # W == 0 via wmask, so any finite value is safe (never NaN).
        def shift_copy(eng, dst, delta):
            dd = abs(delta)
            if delta > 0:
                eng.dma_start(dst[0:128 - dd, :, :], vT[dd:128, :, :])
                eng.dma_start(dst[128 - dd:128, 0:NCHUNK - 1, :],
                              vT[0:dd, 1:NCHUNK, :])
                eng.dma_start(dst[128 - dd:128, NCHUNK - 1, :],
                              vT[0:dd, NCHUNK - 1, :])          # clamp (W=0)
            else:
                eng.dma_start(dst[dd:128, :, :], vT[0:128 - dd, :, :])
                eng.dma_start(dst[0:dd, 1:NCHUNK, :],
                              vT[128 - dd:128, 0:NCHUNK - 1, :])
                eng.dma_start(dst[0:dd, 0, :], vT[128 - dd:128, 0, :])  # clamp

        def vd_tile(tag, nm):
            return main.tile([128, NCHUNK, C], F16, tag=tag, name=nm)

        # A-family first, then edges, then the DVE-side B-family. Tag-reuse
        # WAR waits are always satisfied by readers on OTHER queues.
        # (B_p2/B_m2 are emitted later, between the two PE sweeps.)
        A_p1 = vd_tile("vdd0", "A_p1"); shift_copy(nc.sync, A_p1, 1)
        A_p2 = vd_tile("vdg0", "A_p2"); shift_copy(nc.scalar, A_p2, 2)
        A_m1 = vd_tile("vdd1", "A_m1"); shift_copy(nc.sync, A_m1, -1)
        A_m2 = vd_tile("vdg1", "A_m2"); shift_copy(nc.scalar, A_m2, -2)

        # edge tiles for tap (e=-1, f) at chunk 0: edgeB[p, fi] = v[p - 64 + f]
        # (valid lanes p >= 64; lanes < 64+|f| have W=0, clamp-filled)
        EDGEF = (0, 1, -1, 2, -2)
        FI = {f: i for i, f in enumerate(EDGEF)}
        edgeB = main.tile([128, 5, C], F16, tag="edgeB", name="edgeB")
        for fi, f in enumerate(EDGEF):
            if f >= 0:
                nc.sync.dma_start(edgeB[64:128, fi, :], vT[f:64 + f, 0, :])
                nc.sync.dma_start(edgeB[0:64, fi, :], vT[0:64, 0, :])
            else:
                nc.sync.dma_start(edgeB[64 - f:128, fi, :], vT[0:64 + f, 0, :])
                nc.sync.dma_start(edgeB[0:64 - f, fi, :], vT[0:64 - f, 0, :])

        B_0 = vd_tile("vdd0", "B_0"); shift_copy(nc.sync, B_0, 64)
        B_p1 = vd_tile("vdd1", "B_p1"); shift_copy(nc.sync, B_p1, 65)
        B_m1 = vd_tile("vdd0", "B_m1"); shift_copy(nc.sync, B_m1, 63)
        BF = {0: B_0, 1: B_p1, -1: B_m1}

        # ---- phase E: 25-tap apply ----
        # 13 taps as DVE scalar_tensor_tensor FMAs into acc_d; 12 taps on
        # ScalarE+PE: ScalarE builds diag(w) tiles (reads only ident/W_tm),
        # PE accumulates psum[j] += diag(w) @ v_shifted into per-chunk PSUM
        # banks, folding acc_d in at the end.  GPSIMD is useless here -- it
        # contends with DVE's SBUF port and its AP-scalar ops run at ~4us.
        acc_d = main.tile([128, NCHUNK, C], F16, tag="acc", name="acc_d")
        acc2 = main.tile([128, NCHUNK, C], F16, tag="accg", name="acc2")
        VDT = {0: vT, 1: A_p1, -1: A_m1, 2: A_p2, -2: A_m2}

        def tap_sources(e, f):
            """Yield (j, src_ap) for tap (e, f)."""
            if e % 2 == 0:
                vdt, off = VDT[f], e // 2
                for j in range(NCHUNK):
                    jp = j + off
                    if 0 <= jp < NCHUNK:
                        yield j, vdt[:, jp, :]
            elif e == 1:
                bt = BF[f]
                for j in range(NCHUNK):
                    yield j, bt[:, j, :]
            else:
                bt = BF[f]
                yield 0, edgeB[:, FI[f], :]
                for j in range(1, NCHUNK):
                    yield j, bt[:, j - 1, :]

        def dve_fma(j, src, t, first=False):
            wap = W_tm[:, j:j + 1, t:t + 1]
            dst = acc_d[:, j, :]
            if first:
                nc.vector.tensor_scalar(dst, src, wap, None, op0=ALU.mult)
            else:
                nc.vector.scalar_tensor_tensor(dst, src, wap, dst,
                                               op0=ALU.mult, op1=ALU.add)

        # DVE phase 1 (chunk-inner; sources available early)
        for j in range(NCHUNK):
            dve_fma(j, vT[:, j, :], TAPI[(0, 0)], first=True)
        # A_p1-sourced taps first, then A_m1 -- releases those tags early so
        # the B-family shift-copy DMAs (tag WAR) start while phase 1 runs;
        # the vT-sourced (+-2, 0) taps last cover the DMA latency.
        for e, f in ((0, 1), (2, 1), (-2, 1), (0, -1), (2, -1), (-2, -1),
                     (2, 0), (-2, 0)):
            t = TAPI[(e, f)]
            for j, src in tap_sources(e, f):
                dve_fma(j, src, t)
        # DVE phase 2 (tap-major; B-family sources arrive while phase 1 runs)
        for e, f in ((1, 0), (-1, 0), (1, 1), (-1, 1), (1, -1), (-1, -1)):
            t = TAPI[(e, f)]
            for j, src in tap_sources(e, f):
                dve_fma(j, src, t)

        # ScalarE+PE path
        dtp = es.enter_context(tc.tile_pool(name="dtp", bufs=4))
        cmE = tc.tile_pool(name="psE", bufs=1, space="PSUM"); psE = cmE.__enter__()
        GRP = 8
        SWEEP_A = [(0, 2), (2, 2), (-2, 2), (0, -2), (2, -2), (-2, -2)]
        SWEEP_B = [(1, 2), (-1, 2), (1, -2), (-1, -2)]

        def pe_sweep(taps, fold_acc2, fold_accd):
            for g0 in range(0, NCHUNK, GRP):
                ops = {j: [] for j in range(g0, g0 + GRP)}
                for e, f in taps:
                    t = TAPI[(e, f)]
                    for j, src in tap_sources(e, f):
                        if g0 <= j < g0 + GRP:
                            ops[j].append((t, src))
                for j in range(g0, g0 + GRP):
                    if fold_acc2:
                        ops[j].append((None, acc2[:, j, :]))
                    if fold_accd:
                        ops[j].append((None, acc_d[:, j, :]))
                pss = {j: psE.tile([128, 512], F32, tag=f"eps{j - g0}",
                                   name=f"eps{j - g0}")
                       for j in range(g0, g0 + GRP)}
                for j in range(g0, g0 + GRP):
                    n = len(ops[j])
                    for k, (t, src) in enumerate(ops[j]):
                        if t is None:
                            lhs = ident[:]
                        else:
                            dt = dtp.tile([128, 128], F16, tag="dt", name="dt")
                            nc.scalar.activation(dt[:], ident[:], ACT.Copy,
                                                 scale=W_tm[:, j:j + 1, t:t + 1])
                            lhs = dt[:]
                        nc.tensor.matmul(pss[j][:, 0:C], lhs, src,
                                         start=(k == 0), stop=(k == n - 1))
                    nc.scalar.copy(acc2[:, j, :], pss[j][:, 0:C])

        # sweep A: 6 even-e taps -> acc2 (runs alongside DVE phase 1)
        pe_sweep(SWEEP_A, fold_acc2=False, fold_accd=False)
        # B_p2/B_m2 copies: emitted here so their WAR waits (on sweep A's
        # PE-side A_p2/A_m2 reads) sit ahead of only sweep-B work
        B_p2 = vd_tile("vdg0", "B_p2"); shift_copy(nc.scalar, B_p2, 66)
        B_m2 = vd_tile("vdg1", "B_m2"); shift_copy(nc.scalar, B_m2, 62)
        BF.update({2: B_p2, -2: B_m2})
        # sweep B: 4 odd-e taps + acc2 -> acc2 (independent of acc_d)
        pe_sweep(SWEEP_B, fold_acc2=True, fold_accd=False)
        cmE.__exit__(None, None, None)
        # final merge on DVE right after its last tap (fp16 2x mode)
        for j in range(NCHUNK):
            nc.vector.tensor_tensor(acc_d[:, j, :], acc_d[:, j, :],
                                    acc2[:, j, :], op=ALU.add)
        if debug:
            nc.gpsimd.dma_start(
                d["dbg_acc"].rearrange("p (j c) -> p j c", j=NCHUNK), acc_d[:])

        # ---- phase G: xf transpose-evac + relu/maxpool chain ----
        # x1 = relu(relu(xfT) + m1^T); x2 = relu(x1 + m2^T)  (x2 in-place in m2;
        # x1 reuses the DVE vd slots, which are dead after phase E)
        cmG = tc.tile_pool(name="psG", bufs=4, space="PSUM"); psG = cmG.__enter__()
        x1 = [main.tile([128, L], F16, tag=f"vdd{cc}", name=f"x1_{cc}")
              for cc in range(2)]
        for j2 in range(NCHUNK // 2):
            for cc in range(2):
                pt = psG.tile([128, 2, 128], F16, tag="tp", name="tp")
                for u in range(2):
                    nc.tensor.transpose(
                        pt[:, u, :],
                        acc_d[:, 2 * j2 + u, 128 * cc:128 * (cc + 1)], ident[:])
                nc.scalar.activation(x1[cc][:, 256 * j2:256 * (j2 + 1)],
                                     pt.rearrange("p a b -> p (a b)"), ACT.Relu)
        cmG.__exit__(None, None, None)
        x2 = m2
        for cc in range(2):
            nc.vector.tensor_tensor(x1[cc][:], x1[cc][:], m1[cc][:], op=ALU.add)
            nc.scalar.activation(x1[cc][:], x1[cc][:], ACT.Relu)
            nc.vector.tensor_tensor(x2[cc][:], x1[cc][:], m2[cc][:], op=ALU.add)
            nc.scalar.activation(x2[cc][:], x2[cc][:], ACT.Relu)
        if debug:
            for cc in range(2):
                nc.gpsimd.dma_start(d["dbg_x1"][128 * cc:128 * (cc + 1), :], x1[cc][:])

        # ---- phase H: fu matmul + bias + relu + residual, incremental BN ----
        cmH = tc.tile_pool(name="psH", bufs=2, space="PSUM"); psH = cmH.__enter__()
        out_all = main.tile([128, 2, L], F16, tag="acc", name="out_all")
        out_cm = [out_all[:, cc, :] for cc in range(2)]
        st = small.tile([128, 2, 8, 6], F32, tag="bnst", name="bnst")
        rhss = [x1[0], x1[1], x2[0], x2[1]]
        for mc in range(2):
            for half in range(2):
                ps = psH.tile([128, 4, 512], F32, tag="fups", name="fups")
                for q in range(4):
                    n8 = 4 * half + q
                    for kc in range(4):
                        nc.tensor.matmul(ps[:, q, :], wfu_sb[:, kc, mc, :],
                                         rhss[kc][:, 512 * n8:512 * (n8 + 1)],
                                         start=(kc == 0), stop=False)
                    nc.tensor.matmul(ps[:, q, :],
                                     bfu_row[:, 128 * mc:128 * (mc + 1)],
                                     onesr[:], start=False, stop=True)
                for q in range(4):
                    n8 = 4 * half + q
                    sl = slice(512 * n8, 512 * (n8 + 1))
                    nc.scalar.activation(out_cm[mc][:, sl], ps[:, q, :], ACT.Relu)
                    nc.vector.tensor_tensor(out_cm[mc][:, sl], out_cm[mc][:, sl],
                                            xT_cm[mc][:, sl], op=ALU.add)
                    nc.vector.bn_stats(st[:, mc, n8, :], out_cm[mc][:, sl])
        cmH.__exit__(None, None, None)

        # ---- BN: pack local sums, single AllReduce, normalize ----
        bnpack = small.tile([128, 4], F32, tag="bnpack", name="bnpack")
        for mc in range(2):
            ag = small.tile([128, 2], F32, tag="bnag", name="bnag", bufs=2)
            nc.vector.bn_aggr(ag[:], st[:, mc])
            nc.vector.tensor_scalar(bnpack[:, 2 * mc:2 * mc + 1], ag[:, 0:1],
                                    float(L), None, op0=ALU.mult)
            sq = small.tile([128, 1], F32, tag="bnsq", name="bnsq", bufs=2)
            nc.vector.tensor_tensor(sq[:], ag[:, 0:1], ag[:, 0:1], op=ALU.mult)
            nc.vector.tensor_tensor(sq[:], sq[:], ag[:, 1:2], op=ALU.add)
            nc.vector.tensor_scalar(bnpack[:, 2 * mc + 1:2 * mc + 2], sq[:],
                                    float(L), None, op0=ALU.mult)
        if HOST_BN:
            # ship local sums + unnormalized activations; host finishes BN
            nc.sync.dma_start(d["stats"][:], bnpack[:])
            for cc in range(2):
                for hh in range(2):
                    sl = slice(2048 * hh, 2048 * (hh + 1))
                    nc.sync.dma_start(d["y"][128 * cc:128 * (cc + 1), sl],
                                      out_cm[cc][:, sl])
        else:
            cin = dram.tile([128, 4], F32, name="cin")
            cout = dram.tile([128, 4], F32, name="cout")
            nc.sync.dma_start(cin[:], bnpack[:])
            nc.gpsimd.collective_compute(
                "AllReduce", ALU.add, replica_groups=[list(range(n_cores))],
                ins=[cin.opt()], outs=[cout.opt()])
            gs = small.tile([128, 4], F32, tag="gs", name="gs")
            nc.sync.dma_start(gs[:], cout[:])

            NTOT = float(n_cores * L)
            scale = small.tile([128, 2], F32, tag="scale", name="scale")
            shift = small.tile([128, 2], F32, tag="shift", name="shift")
            mean = small.tile([128, 2], F32, tag="mean", name="mean")
            var = small.tile([128, 2], F32, tag="var", name="var")
            for cc in range(2):
                nc.vector.tensor_scalar(mean[:, cc:cc + 1], gs[:, 2 * cc:2 * cc + 1],
                                        1.0 / NTOT, None, op0=ALU.mult)
                nc.vector.tensor_scalar(var[:, cc:cc + 1], gs[:, 2 * cc + 1:2 * cc + 2],
                                        1.0 / NTOT, None, op0=ALU.mult)
            msq = small.tile([128, 2], F32, tag="msq", name="msq")
            nc.vector.tensor_tensor(msq[:], mean[:], mean[:], op=ALU.mult)
            nc.vector.tensor_tensor(var[:], var[:], msq[:], op=ALU.subtract)
            rs = small.tile([128, 2], F32, tag="rs", name="rs")
            nc.vector.tensor_scalar(var[:], var[:], float(EPS), None, op0=ALU.add)
            nc.scalar.activation(rs[:], var[:], ACT.Sqrt)
            nc.vector.reciprocal(rs[:], rs[:])
            nc.vector.tensor_tensor(scale[:], gamma2[:], rs[:], op=ALU.mult)
            nc.vector.tensor_tensor(shift[:], mean[:], scale[:], op=ALU.mult)
            nc.vector.tensor_tensor(shift[:], beta2[:], shift[:], op=ALU.subtract)

            for cc in range(2):
                for hh in range(2):
                    sl = slice(2048 * hh, 2048 * (hh + 1))
                    nc.vector.tensor_scalar(out_cm[cc][:, sl], out_cm[cc][:, sl],
                                            scale[:, cc:cc + 1], shift[:, cc:cc + 1],
                                            op0=ALU.mult, op1=ALU.add)
                    nc.sync.dma_start(d["y"][128 * cc:128 * (cc + 1), sl],
                                      out_cm[cc][:, sl])


_CACHE = {}


def _get_program(n_cores=N_CORES, debug=False):
    key = (n_cores, debug)
    if key not in _CACHE:
        nc = bacc.Bacc("TRN2", target_bir_lowering=False, debug=False,
                       num_devices=n_cores)
        build(nc, n_cores, debug)
        nc.compile()
        _CACHE[key] = nc
    return _CACHE[key]


def make_in_map(inputs, b):
    consts = host_consts()
    f16 = np.float16
    # host-side grid permutation: xf16[u*64+v, c] = x[v, u, c]
    xf16 = np.ascontiguousarray(
        np.asarray(inputs["x"][b]).transpose(1, 0, 2).reshape(L, C)).astype(f16)
    wv = np.ascontiguousarray(
        np.asarray(inputs["Wv"], np.float32).reshape(2, 128, C)
        .transpose(1, 0, 2).reshape(128, 2 * C)).astype(f16)
    wa = np.ascontiguousarray(
        np.asarray(inputs["Wa"], np.float32).reshape(2, 128, 81)
        .transpose(1, 0, 2).reshape(128, 2 * 81)).astype(f16)
    wfu = np.ascontiguousarray(
        np.asarray(inputs["Wfu"], np.float32).reshape(4, 128, 2, 128)
        .transpose(1, 0, 2, 3).reshape(128, 4 * 2 * 128)).astype(f16)
    return {
        "xf16": xf16,
        "wv": wv, "wa": wa, "wfu": wfu,
        "bvrow": np.asarray(inputs["bv"], np.float32).reshape(1, C).astype(f16),
        "barow": np.ascontiguousarray(
            np.asarray(inputs["ba"], np.float32).reshape(81, 1)),
        "bfurow": np.asarray(inputs["bfu"], np.float32).reshape(1, C).astype(f16),
        "gamma2": np.ascontiguousarray(
            np.asarray(inputs["gamma"], np.float32).reshape(2, 128).T),
        "beta2": np.ascontiguousarray(
            np.asarray(inputs["beta"], np.float32).reshape(2, 128).T),
        **consts,
    }


def gather_full(results, inputs):
    # y[c, u*64+v] -> out[u, v, c], cast fp16 -> f32 on host; with HOST_BN the
    # global batch-norm (exact, all 8 cores' stats) is applied here.
    ys = np.stack([np.asarray(results[b]["y"], dtype=np.float32)
                   for b in range(B)])                      # [B, C, L]
    if HOST_BN:
        st = np.stack([np.asarray(results[b]["stats"], dtype=np.float64)
                       for b in range(B)])                  # [B, 128, 4]
        st = st.sum(axis=0)
        s_pack = st.reshape(128, 2, 2)                      # [p, cc, (sum, sumsq)]
        cnt = float(B * L)
        mean = (s_pack[:, :, 0].T.reshape(C) / cnt)         # [C] (cc-major)
        ex2 = (s_pack[:, :, 1].T.reshape(C) / cnt)
        var = ex2 - mean * mean
        gamma = np.asarray(inputs["gamma"], np.float64)
        beta = np.asarray(inputs["beta"], np.float64)
        scale = gamma / np.sqrt(var + EPS)
        shift = beta - mean * scale
        ys = ys * scale[None, :, None] + shift[None, :, None]
    out = ys.reshape(B, C, H, W).transpose(0, 2, 3, 1)
    return np.ascontiguousarray(out, dtype=np.float32)


def kernel(**inputs):
    nc = _get_program()
    in_maps = [make_in_map(inputs, b) for b in range(B)]
    res = run_bass_kernel_spmd(nc, in_maps, list(range(N_CORES)))
    return gather_full(res.results, inputs)


# revision 28
# speedup vs baseline: 1.4200x; 1.0335x over previous
"""Trainium2 Bass kernel for nn_MOA_13254269075617 (sparse windowed attention block).

Sharding: data-parallel over batch B=8 across 8 NeuronCores (1 image each).
BatchNorm uses global batch stats via one on-device AllReduce of per-channel
sum / sum-of-squares (plus an early warm-up collective to absorb CC-stream
startup cost).

Per-core pipeline (all in the spatially-TRANSPOSED frame T(z)[u,v]=z[v,u],
fp16 end-to-end so DVE elementwise ops run in 2x/4x perf modes):
  xT_cm  : x in channel-major [256, 4096], host-pre-permuted grid, loaded
           via 4 big transpose-DMAs
  vT     : (xT @ Wv + bv) token-major [128, 32, 256] fp16
  AE     : exp-logits pq-major [81, 4096] on a zero-padded 66-pitch grid
  W      : 25-tap position-varying stencil weights (fold+attention combined)
  vd     : 9 token-shifted copies of vT via SBUF->SBUF DMA (no PE work)
  acc    : 25-tap stencil apply, token-major fp16 FMAs on DVE (2x mode)
  x1/x2  : relu chains with 3x3/5x5 maxpools (separable shifted-max trees)
  out    : concat-matmul (Wfu) + residual, BN with AllReduce'd stats
  y      : channel-major fp16 output, host casts/transposes to f32 [H,W,C]
"""
import sys

for _p in (
    "/root/.axon_site",
    "/root/.axon_site/_ro/trn_rl_repo",
    "/root/.axon_site/_ro/pypackages",
    "/opt/trn_rl_repo",
):
    if _p not in sys.path:
        sys.path.append(_p)

from itertools import product

import numpy as np

import concourse.bass as bass
import concourse.tile as tile
from concourse import bacc, mybir
from concourse.bass_utils import run_bass_kernel_spmd

F32 = mybir.dt.float32
F16 = mybir.dt.float16
ALU = mybir.AluOpType
ACT = mybir.ActivationFunctionType

B, H, W, C = 8, 64, 64, 256
HOST_BN = True   # cross-core BN reduction on host (hint allows local stats;
                 # this is exact) -- removes both collectives + entry barrier
L = H * W                      # 4096 tokens
NCHUNK = L // 128              # 32 token chunks
N_CORES = 8
EPS = 1e-5

# (e, f) tap -> t index and source shift delta = 64*e + f
TAPI = {(e, f): (e + 2) * 5 + (f + 2) for e in range(-2, 3) for f in range(-2, 3)}


def host_consts():
    """Selector matrices and small constants (host-precomputed, same all cores)."""
    selsum = np.zeros((81, 9), np.float32)
    for p in range(9):
        selsum[9 * p:9 * p + 9, p] = 1.0
    selrep = np.zeros((9, 81), np.float32)
    for p in range(9):
        selrep[p, 9 * p:9 * p + 9] = 1.0
    # selshift[:, 25*d + tap]: for (di,dj) block d, tap (e,f):
    #   k = 9*(3di+dj) + 3(di+e)+(dj+f) if di+e,dj+f in [0,3)
    selshift = np.zeros((81, 9 * 25), np.float32)
    for d, (di, dj) in enumerate(product(range(3), range(3))):
        for t, (e, f) in enumerate(product(range(-2, 3), range(-2, 3))):
            dip, djp = di + e, dj + f
            if 0 <= dip < 3 and 0 <= djp < 3:
                k = 9 * (3 * di + dj) + (3 * dip + djp)
                selshift[k, 25 * d + t] = 1.0
    wmask = np.ones((25, 64, 64), np.float32)
    for t, (e, f) in enumerate(product(range(-2, 3), range(-2, 3))):
        if e > 0: wmask[t, 64 - e:, :] = 0
        if e < 0: wmask[t, :-e, :] = 0
        if f > 0: wmask[t, :, 64 - f:] = 0
        if f < 0: wmask[t, :, :-f] = 0
    f16 = np.float16
    return {
        "selsum": selsum.astype(f16),
        "selrep": selrep.astype(f16),
        "selshift": selshift.astype(f16),
        "wmask": wmask.reshape(25, 4096).astype(f16),
        "ident": np.eye(128, dtype=f16),
        "onesr": np.ones((1, 512), f16),
    }


def build(nc, n_cores, debug=False):
    d = {}
    def din(name, shape, dt=F16):
        d[name] = nc.dram_tensor(name, list(shape), dt, kind="ExternalInput").ap()
    def dout(name, shape, dt=F16):
        d[name] = nc.dram_tensor(name, list(shape), dt, kind="ExternalOutput").ap()

    din("xf16", (L, C))
    din("wv", (128, 2 * C)); din("bvrow", (1, C))
    din("wa", (128, 2 * 81)); din("barow", (81, 1), F32)
    din("wfu", (128, 4 * 2 * 128)); din("bfurow", (1, C))
    din("gamma2", (128, 2), F32); din("beta2", (128, 2), F32)
    din("selsum", (81, 9)); din("selrep", (9, 81)); din("selshift", (81, 225))
    din("ident", (128, 128)); din("onesr", (1, 512)); din("wmask", (25, L))
    dout("y", (2 * 128, L))
    dout("stats", (128, 4), F32)
    if debug:
        dout("dbg_vt", (128, NCHUNK * C))
        dout("dbg_w", (25, L))
        dout("dbg_acc", (128, NCHUNK * C))
        dout("dbg_x1", (2 * 128, L))

    with tile.TileContext(nc) as tc:
        _build_tc(tc, d, n_cores, debug)
    return d


def _build_tc(tc, d, n_cores, debug):
    nc = tc.nc
    from contextlib import ExitStack
    es = ExitStack()
    with es:
        consts = es.enter_context(tc.tile_pool(name="consts", bufs=1))
        main = es.enter_context(tc.tile_pool(name="main", bufs=1))
        small = es.enter_context(tc.tile_pool(name="small", bufs=1))
        dram = es.enter_context(tc.tile_pool(name="dram", bufs=2, space="DRAM"))

        # ---- phase A: input loads first (run under the CC entry barrier) ----
        xT_cm = [main.tile([128, L], F16, tag=f"xcm{cc}", name=f"xT_cm{cc}")
                 for cc in range(2)]
        for cc in range(2):
            nc.sync.dma_start_transpose(
                xT_cm[cc][:, :], d["xf16"][:, 128 * cc:128 * (cc + 1)])

        # ---- const loads on the scalar HWDGE queue ----
        def cload(name, shape, dt=F16, src=None):
            t = consts.tile(list(shape), dt, tag=name, name=name)
            nc.scalar.dma_start(t[:], (src if src is not None else d[name])[:])
            return t
        ident = cload("ident", (128, 128))
        onesr = cload("onesr", (1, 512))
        bv_row = cload("bvrow", (1, C))
        bfu_row = cload("bfurow", (1, C))
        ba_sb = cload("barow", (81, 1), F32)
        selsum = cload("selsum", (81, 9))
        selrep = cload("selrep", (9, 81))
        selshift = cload("selshift", (81, 225))
        gamma2 = cload("gamma2", (128, 2), F32)
        beta2 = cload("beta2", (128, 2), F32)
        wv_sb = cload("wv", (128, 2, C), src=d["wv"].rearrange("p (k c) -> p k c", k=2))
        wa_sb = cload("wa", (128, 2, 81), src=d["wa"].rearrange("p (k c) -> p k c", k=2))
        wfu_sb = cload("wfu", (128, 4, 2, 128),
                       src=d["wfu"].rearrange("p (k m c) -> p k m c", k=4, m=2))

        if not HOST_BN:
            # warm-up collective: absorbs CC stream startup (~20us) early
            warm_in = dram.tile([1, 1], F32, name="warm_in")
            warm_out = dram.tile([1, 1], F32, name="warm_out")
            nc.sync.dma_start(warm_in[:], ba_sb[0:1, 0:1])
            nc.gpsimd.collective_compute(
                "AllReduce", ALU.add, replica_groups=[list(range(n_cores))],
                ins=[warm_in.opt()], outs=[warm_out.opt()])

        # ---- phase C: attention logits -> exp -> normalize ----
        # AE grid: (g1=u, g2=v); AE[g1+1, g2+1] = softmax-numerator of the
        # ORIGINAL position (h=g2, w=g1) (x transposed-grid ordering).
        cmMid = tc.tile_pool(name="mid", bufs=1); mid = cmMid.__enter__()
        wmask = mid.tile([25, L], F16, tag="wmask", name="wmask")
        nc.scalar.dma_start(wmask[:], d["wmask"][:])
        cmCl = tc.tile_pool(name="psClog", bufs=1, space="PSUM"); psCl = cmCl.__enter__()
        AE = mid.tile([81, 66 * 67], F16, tag="AE", name="AE")
        AE3 = AE.rearrange("p (r s) -> p r s", r=67)
        nc.vector.memset(AE3[:, 0:1, :], 0.0)          # pad ring only; the
        nc.vector.memset(AE3[:, 65:67, :], 0.0)        # 64x64 interior is
        nc.vector.memset(AE3[:, 1:65, 0:1], 0.0)       # fully written by exp
        nc.vector.memset(AE3[:, 1:65, 65:66], 0.0)

        for grp in range(2):
            pss = [psCl.tile([81, 512], F32, tag=f"aps{q}", name=f"aps{q}")
                   for q in range(4)]
            for kc in range(2):
                for q in range(4):
                    n8 = 4 * grp + q
                    nc.tensor.matmul(pss[q][:], wa_sb[:, kc, :],
                                     xT_cm[kc][:, 512 * n8:512 * (n8 + 1)],
                                     start=(kc == 0), stop=(kc == 1))
            for q in range(4):
                n8 = 4 * grp + q
                nc.scalar.activation(AE3[:, 1 + 8 * n8:1 + 8 * n8 + 8, 1:65],
                                     pss[q].rearrange("p (r s) -> p r s", s=64),
                                     ACT.Exp, bias=ba_sb[:, 0:1])

        # ---- phase B: vT = xT @ Wv + bv, token-major fp16 ----
        # (emitted between C-logits and C-rowsums so vT lands early for the
        # shifted-copy DMAs; PE queue order = emission order)
        vT = main.tile([128, NCHUNK, C], F16, tag="vT", name="vT")
        cmB = tc.tile_pool(name="psB", bufs=2, space="PSUM"); psB = cmB.__enter__()
        for g in range(8):
            ps = psB.tile([128, 4, C], F32, tag="bps", name="bps")
            for jj in range(4):
                j = 4 * g + jj
                for kc in range(2):
                    nc.tensor.matmul(ps[:, jj, :],
                                     xT_cm[kc][:, 128 * j:128 * (j + 1)],
                                     wv_sb[:, kc, :], start=(kc == 0), stop=False)
                nc.tensor.matmul(ps[:, jj, :], onesr[:, 0:128], bv_row[:],
                                 start=False, stop=True)
            for jj in range(4):
                nc.scalar.copy(vT[:, 4 * g + jj, :], ps[:, jj, :])
        cmB.__exit__(None, None, None)
        cmCl.__exit__(None, None, None)
        if debug:
            nc.gpsimd.dma_start(
                d["dbg_vt"].rearrange("p (j c) -> p j c", j=NCHUNK), vT[:])

        # ---- phase C (cont.): per-p row sums via selector matmuls ----
        cmC = tc.tile_pool(name="psCrow", bufs=2, space="PSUM"); psC = cmC.__enter__()
        ROWCH = [(r0, min(7, 64 - r0)) for r0 in range(0, 64, 7)]
        for r0, nr in ROWCH:
            N = nr * 66
            win = slice((r0 + 1) * 66, (r0 + 1) * 66 + N)
            ps = psC.tile([9, 512], F32, tag="sps", name="sps")
            nc.tensor.matmul(ps[:, 0:N], selsum[:], AE[:, win],
                             start=True, stop=True)
            rchf = small.tile([9, 512], F32, tag="rchf", name="rchf", bufs=2)
            nc.vector.reciprocal_approx_fast(rchf[:, 0:N], ps[:, 0:N])
            rch = small.tile([9, 512], F16, tag="rch", name="rch", bufs=2)
            nc.scalar.copy(rch[:, 0:N], rchf[:, 0:N])
            ps2 = psC.tile([81, 512], F32, tag="rps", name="rps")
            nc.tensor.matmul(ps2[:, 0:N], selrep[:], rch[:, 0:N],
                             start=True, stop=True)
            rt = small.tile([81, 512], F16, tag="rt", name="rt", bufs=2)
            nc.scalar.copy(rt[:, 0:N], ps2[:, 0:N])
            iv = AE3[:, r0 + 1:r0 + 1 + nr, 1:65]
            nc.vector.tensor_tensor(
                iv, iv, rt[:, 0:N].rearrange("p (r s) -> p r s", s=66)[:, :, 1:65],
                op=ALU.mult)
        cmC.__exit__(None, None, None)

        # ---- phase D: W stencil build (9 shifted selector matmuls) ----
        cmD = tc.tile_pool(name="psD", bufs=1, space="PSUM"); psD = cmD.__enter__()
        W_tap = mid.tile([25, L], F16, tag="wtap", name="W_tap")
        wmask_t = wmask.rearrange("p (u v) -> p v u", u=64)
        wtap_t = W_tap.rearrange("p (u v) -> p v u", u=64)
        for gstart in (0, 5):
            grp = ROWCH[gstart:gstart + 5]
            pss = [psD.tile([25, 512], F32, tag=f"wps{q}", name=f"wps{q}")
                   for q in range(len(grp))]
            for dd, (di, dj) in enumerate(product(range(3), range(3))):
                for q, (r0, nr) in enumerate(grp):
                    N = nr * 66
                    st = (r0 + 2 - dj) * 66 + (2 - di)
                    nc.tensor.matmul(pss[q][:, 0:N],
                                     selshift[:, 25 * dd:25 * (dd + 1)],
                                     AE[:, st:st + N],
                                     start=(dd == 0), stop=(dd == 8))
            for q, (r0, nr) in enumerate(grp):
                N = nr * 66
                wt = small.tile([25, 512], F16, tag="wt", name="wt", bufs=2)
                nc.scalar.copy(wt[:, 0:N], pss[q][:, 0:N])
                nc.vector.tensor_tensor(
                    wtap_t[:, r0:r0 + nr, :],
                    wt[:, 0:N].rearrange("p (r s) -> p r s", s=66)[:, :, 0:64],
                    wmask_t[:, r0:r0 + nr, :], op=ALU.mult)
        if debug:
            nc.gpsimd.dma_start(d["dbg_w"][:], W_tap[:])
        cmD.__exit__(None, None, None)
        cmD2 = tc.tile_pool(name="psD2", bufs=2, space="PSUM"); psD2 = cmD2.__enter__()
        W_tm = main.tile([128, NCHUNK, 25], F32, tag="W_tm", name="W_tm")
        for j in range(NCHUNK):
            pt = psD2.tile([128, 25], F16, tag="wtp", name="wtp")
            nc.tensor.transpose(pt[:], W_tap[:, 128 * j:128 * (j + 1)],
                                ident[0:25, 0:25])
            nc.scalar.copy(W_tm[:, j, :], pt[:])
        cmD2.__exit__(None, None, None)
        cmMid.__exit__(None, None, None)

        # ---- maxpools first in the DVE stream (only need xT_cm; they fill
        # the head while W/vd are still being built) ----
        ptmp = es.enter_context(tc.tile_pool(name="ptmp", bufs=1))
        m1 = [main.tile([128, L], F16, tag=f"m1{cc}", name=f"m1_{cc}") for cc in range(2)]
        m2 = [main.tile([128, L], F16, tag=f"m2{cc}", name=f"m2_{cc}") for cc in range(2)]

        def g3(ap):
            return ap.rearrange("p (h w) -> p h w", h=64)

        def hmax3(eng, dst, src):
            dv, sv = g3(dst), g3(src)
            t1 = ptmp.tile([128, L], F16, tag="ptmp", name="ptmp")
            tv = g3(t1)
            eng.tensor_tensor(tv[:, :, 1:], sv[:, :, 1:], sv[:, :, :63], op=ALU.max)
            nc.scalar.copy(tv[:, :, 0:1], sv[:, :, 0:1])
            eng.tensor_tensor(dv[:, :, :63], tv[:, :, :63], sv[:, :, 1:], op=ALU.max)
            nc.scalar.copy(dv[:, :, 63:64], tv[:, :, 63:64])

        def vmax3(eng, dst, src):
            dv, sv = g3(dst), g3(src)
            t1 = ptmp.tile([128, L], F16, tag="ptmp", name="ptmp")
            tv = g3(t1)
            eng.tensor_tensor(tv[:, 1:, :], sv[:, 1:, :], sv[:, :63, :], op=ALU.max)
            nc.scalar.copy(tv[:, 0:1, :], sv[:, 0:1, :])
            eng.tensor_tensor(dv[:, :63, :], tv[:, :63, :], sv[:, 1:, :], op=ALU.max)
            nc.scalar.copy(dv[:, 63:64, :], tv[:, 63:64, :])

        def hspread(eng, dst, src):
            dv, sv = g3(dst), g3(src)
            eng.tensor_tensor(dv[:, :, 1:63], sv[:, :, 0:62], sv[:, :, 2:64], op=ALU.max)
            nc.scalar.copy(dv[:, :, 0:1], sv[:, :, 1:2])
            nc.scalar.copy(dv[:, :, 63:64], sv[:, :, 62:63])

        def vspread(eng, dst, src):
            dv, sv = g3(dst), g3(src)
            eng.tensor_tensor(dv[:, 1:63, :], sv[:, 0:62, :], sv[:, 2:64, :], op=ALU.max)
            nc.scalar.copy(dv[:, 0:1, :], sv[:, 1:2, :])
            nc.scalar.copy(dv[:, 63:64, :], sv[:, 62:63, :])

        for cc in range(2):
            eng = nc.vector
            cm3 = ptmp.tile([128, L], F16, tag="ptmp2", name="ptmp2")
            hmax3(eng, cm3, xT_cm[cc])
            vmax3(eng, m1[cc], cm3)
            cm5 = ptmp.tile([128, L], F16, tag="ptmp3", name="ptmp3")
            hspread(eng, cm5, cm3)
            r35 = ptmp.tile([128, L], F16, tag="ptmp2", name="ptmp2")
            vmax3(eng, r35, cm5)
            vspread(eng, m2[cc], r35)

        # ---- shifted copies of vT via SBUF->SBUF DMA ----
        # S(delta)[p, j] = v[128j + p + delta]; lanes whose source would leave
        # [0, 4096) are clamp-filled with real (finite) data -- their taps have
        # W == 0 via wmask, so any finite value is safe (never NaN).
        def shift_copy(eng, dst, delta):
            dd = abs(delta)
            if delta > 0:
                eng.dma_start(dst[0:128 - dd, :, :], vT[dd:128, :, :])
                eng.dma_start(dst[128 - dd:128, 0:NCHUNK - 1, :],
                              vT[0:dd, 1:NCHUNK, :])
                eng.dma_start(dst[128 - dd:128, NCHUNK - 1, :],
                              vT[0:dd, NCHUNK - 1, :])          # clamp (W=0)
            else:
                eng.dma_start(dst[dd:128, :, :], vT[0:128 - dd, :, :])
                eng.dma_start(dst[0:dd, 1:NCHUNK, :],
                              vT[128 - dd:128, 0:NCHUNK - 1, :])
                eng.dma_start(dst[0:dd, 0, :], vT[128 - dd:128, 0, :])  # clamp

        def vd_tile(tag, nm):
            return main.tile([128, NCHUNK, C], F16, tag=tag, name=nm)

        # A-family first, then edges, then the DVE-side B-family. Tag-reuse
        # WAR waits are always satisfied by readers on OTHER queues.
        # (B_p2/B_m2 are emitted later, between the two PE sweeps.)
        A_p1 = vd_tile("vdd0", "A_p1"); shift_copy(nc.sync, A_p1, 1)
        A_p2 = vd_tile("vdg0", "A_p2"); shift_copy(nc.scalar, A_p2, 2)
        A_m1 = vd_tile("vdd1", "A_m1"); shift_copy(nc.sync, A_m1, -1)
        A_m2 = vd_tile("vdg1", "A_m2"); shift_copy(nc.scalar, A_m2, -2)

        # edge tiles for tap (e=-1, f) at chunk 0: edgeB[p, fi] = v[p - 64 + f]
        # (valid lanes p >= 64; lanes < 64+|f| have W=0, clamp-filled)
        EDGEF = (0, 1, -1, 2, -2)
        FI = {f: i for i, f in enumerate(EDGEF)}
        edgeB = main.tile([128, 5, C], F16, tag="edgeB", name="edgeB")
        for fi, f in enumerate(EDGEF):
            if f >= 0:
                nc.sync.dma_start(edgeB[64:128, fi, :], vT[f:64 + f, 0, :])
                nc.sync.dma_start(edgeB[0:64, fi, :], vT[0:64, 0, :])
            else:
                nc.sync.dma_start(edgeB[64 - f:128, fi, :], vT[0:64 + f, 0, :])
                nc.sync.dma_start(edgeB[0:64 - f, fi, :], vT[0:64 - f, 0, :])

        B_0 = vd_tile("vdd0", "B_0"); shift_copy(nc.sync, B_0, 64)
        B_p1 = vd_tile("vdd1", "B_p1"); shift_copy(nc.sync, B_p1, 65)
        B_p2 = vd_tile("vdg0", "B_p2"); shift_copy(nc.sync, B_p2, 66)
        B_m2 = vd_tile("vdg1", "B_m2"); shift_copy(nc.sync, B_m2, 62)
        B_m1 = vd_tile("vdd0", "B_m1"); shift_copy(nc.sync, B_m1, 63)
        BF = {0: B_0, 1: B_p1, -1: B_m1, 2: B_p2, -2: B_m2}

        # ---- phase E: 25-tap apply ----
        # 13 taps as DVE scalar_tensor_tensor FMAs into acc_d; 12 taps on
        # ScalarE+PE: ScalarE builds diag(w) tiles (reads only ident/W_tm),
        # PE accumulates psum[j] += diag(w) @ v_shifted into per-chunk PSUM
        # banks, folding acc_d in at the end.  GPSIMD is useless here -- it
        # contends with DVE's SBUF port and its AP-scalar ops run at ~4us.
        acc_d = main.tile([128, NCHUNK, C], F16, tag="acc", name="acc_d")
        acc2 = main.tile([128, NCHUNK, C], F16, tag="accg", name="acc2")
        VDT = {0: vT, 1: A_p1, -1: A_m1, 2: A_p2, -2: A_m2}

        def tap_sources(e, f):
            """Yield (j, src_ap) for tap (e, f)."""
            if e % 2 == 0:
                vdt, off = VDT[f], e // 2
                for j in range(NCHUNK):
                    jp = j + off
                    if 0 <= jp < NCHUNK:
                        yield j, vdt[:, jp, :]
            elif e == 1:
                bt = BF[f]
                for j in range(NCHUNK):
                    yield j, bt[:, j, :]
            else:
                bt = BF[f]
                yield 0, edgeB[:, FI[f], :]
                for j in range(1, NCHUNK):
                    yield j, bt[:, j - 1, :]

        def dve_fma(j, src, t, first=False):
            wap = W_tm[:, j:j + 1, t:t + 1]
            dst = acc_d[:, j, :]
            if first:
                nc.vector.tensor_scalar(dst, src, wap, None, op0=ALU.mult)
            else:
                nc.vector.scalar_tensor_tensor(dst, src, wap, dst,
                                               op0=ALU.mult, op1=ALU.add)

        # DVE phase 1 (chunk-inner; sources available early)
        for j in range(NCHUNK):
            dve_fma(j, vT[:, j, :], TAPI[(0, 0)], first=True)
        # A_p1-sourced taps first, then A_m1 -- releases those tags early so
        # the B-family shift-copy DMAs (tag WAR) start while phase 1 runs;
        # the vT-sourced (+-2, 0) taps last cover the DMA latency.
        for e, f in ((0, 1), (2, 1), (-2, 1), (0, -1), (2, -1), (-2, -1),
                     (2, 0), (-2, 0)):
            t = TAPI[(e, f)]
            for j, src in tap_sources(e, f):
                dve_fma(j, src, t)
        # DVE phase 2 (tap-major; B-family sources arrive while phase 1 runs)
        for e, f in ((1, 0), (-1, 0), (1, 1), (-1, 1), (1, -1), (-1, -1)):
            t = TAPI[(e, f)]
            for j, src in tap_sources(e, f):
                dve_fma(j, src, t)

        # ScalarE+PE path
        dtp = es.enter_context(tc.tile_pool(name="dtp", bufs=5))
        cmE = tc.tile_pool(name="psE", bufs=1, space="PSUM"); psE = cmE.__enter__()
        GRP = 8
        SWEEP_A = [(0, 2), (2, 2), (-2, 2), (0, -2), (2, -2), (-2, -2)]
        SWEEP_B = [(1, 2), (-1, 2), (1, -2), (-1, -2)]

        def pe_sweep(taps, fold_acc2, fold_accd):
            for g0 in range(0, NCHUNK, GRP):
                ops = {j: [] for j in range(g0, g0 + GRP)}
                for e, f in taps:
                    t = TAPI[(e, f)]
                    for j, src in tap_sources(e, f):
                        if g0 <= j < g0 + GRP:
                            ops[j].append((t, src))
                for j in range(g0, g0 + GRP):
                    if fold_acc2:
                        ops[j].append((None, acc2[:, j, :]))
                    if fold_accd:
                        ops[j].append((None, acc_d[:, j, :]))
                pss = {j: psE.tile([128, 512], F32, tag=f"eps{j - g0}",
                                   name=f"eps{j - g0}")
                       for j in range(g0, g0 + GRP)}
                for j in range(g0, g0 + GRP):
                    n = len(ops[j])
                    for k, (t, src) in enumerate(ops[j]):
                        if t is None:
                            lhs = ident[:]
                        else:
                            dt = dtp.tile([128, 128], F16, tag="dt", name="dt")
                            nc.scalar.activation(dt[:], ident[:], ACT.Copy,
                                                 scale=W_tm[:, j:j + 1, t:t + 1])
                            lhs = dt[:]
                        nc.tensor.matmul(pss[j][:, 0:C], lhs, src,
                                         start=(k == 0), stop=(k == n - 1))
                    nc.scalar.copy(acc2[:, j, :], pss[j][:, 0:C])

        # sweep A: 6 even-e taps -> acc2 (runs alongside DVE phase 1)
        pe_sweep(SWEEP_A, fold_acc2=False, fold_accd=False)
        # sweep B: 4 odd-e taps + acc2 -> acc2 (independent of acc_d)
        pe_sweep(SWEEP_B, fold_acc2=True, fold_accd=False)
        cmE.__exit__(None, None, None)
        # final merge on DVE right after its last tap (fp16 2x mode)
        for j in range(NCHUNK):
            nc.vector.tensor_tensor(acc_d[:, j, :], acc_d[:, j, :],
                                    acc2[:, j, :], op=ALU.add)
        if debug:
            nc.gpsimd.dma_start(
                d["dbg_acc"].rearrange("p (j c) -> p j c", j=NCHUNK), acc_d[:])

        # ---- phase G: xf transpose-evac + relu/maxpool chain ----
        # x1 = relu(relu(xfT) + m1^T); x2 = relu(x1 + m2^T)  (x2 in-place in m2;
        # x1 reuses the DVE vd slots, which are dead after phase E)
        cmG = tc.tile_pool(name="psG", bufs=4, space="PSUM"); psG = cmG.__enter__()
        x1 = [main.tile([128, L], F16, tag=f"vdd{cc}", name=f"x1_{cc}")
              for cc in range(2)]
        for j2 in range(NCHUNK // 2):
            for cc in range(2):
                pt = psG.tile([128, 2, 128], F16, tag="tp", name="tp")
                for u in range(2):
                    nc.tensor.transpose(
                        pt[:, u, :],
                        acc_d[:, 2 * j2 + u, 128 * cc:128 * (cc + 1)], ident[:])
                nc.scalar.activation(x1[cc][:, 256 * j2:256 * (j2 + 1)],
                                     pt.rearrange("p a b -> p (a b)"), ACT.Relu)
        cmG.__exit__(None, None, None)
        x2 = m2
        for cc in range(2):
            nc.vector.tensor_tensor(x1[cc][:], x1[cc][:], m1[cc][:], op=ALU.add)
            nc.scalar.activation(x1[cc][:], x1[cc][:], ACT.Relu)
            nc.vector.tensor_tensor(x2[cc][:], x1[cc][:], m2[cc][:], op=ALU.add)
            nc.scalar.activation(x2[cc][:], x2[cc][:], ACT.Relu)
        if debug:
            for cc in range(2):
                nc.gpsimd.dma_start(d["dbg_x1"][128 * cc:128 * (cc + 1), :], x1[cc][:])

        # ---- phase H: fu matmul + bias + relu + residual, incremental BN ----
        cmH = tc.tile_pool(name="psH", bufs=2, space="PSUM"); psH = cmH.__enter__()
        out_all = main.tile([128, 2, L], F16, tag="acc", name="out_all")
        out_cm = [out_all[:, cc, :] for cc in range(2)]
        st = small.tile([128, 2, 8, 6], F32, tag="bnst", name="bnst")
        rhss = [x1[0], x1[1], x2[0], x2[1]]
        for mc in range(2):
            for half in range(2):
                ps = psH.tile([128, 4, 512], F32, tag="fups", name="fups")
                for q in range(4):
                    n8 = 4 * half + q
                    for kc in range(4):
                        nc.tensor.matmul(ps[:, q, :], wfu_sb[:, kc, mc, :],
                                         rhss[kc][:, 512 * n8:512 * (n8 + 1)],
                                         start=(kc == 0), stop=False)
                    nc.tensor.matmul(ps[:, q, :],
                                     bfu_row[:, 128 * mc:128 * (mc + 1)],
                                     onesr[:], start=False, stop=True)
                for q in range(4):
                    n8 = 4 * half + q
                    sl = slice(512 * n8, 512 * (n8 + 1))
                    nc.scalar.activation(out_cm[mc][:, sl], ps[:, q, :], ACT.Relu)
                    nc.vector.tensor_tensor(out_cm[mc][:, sl], out_cm[mc][:, sl],
                                            xT_cm[mc][:, sl], op=ALU.add)
                    nc.vector.bn_stats(st[:, mc, n8, :], out_cm[mc][:, sl])
        cmH.__exit__(None, None, None)

        # ---- BN: pack local sums, single AllReduce, normalize ----
        bnpack = small.tile([128, 4], F32, tag="bnpack", name="bnpack")
        for mc in range(2):
            ag = small.tile([128, 2], F32, tag="bnag", name="bnag", bufs=2)
            nc.vector.bn_aggr(ag[:], st[:, mc])
            nc.vector.tensor_scalar(bnpack[:, 2 * mc:2 * mc + 1], ag[:, 0:1],
                                    float(L), None, op0=ALU.mult)
            sq = small.tile([128, 1], F32, tag="bnsq", name="bnsq", bufs=2)
            nc.vector.tensor_tensor(sq[:], ag[:, 0:1], ag[:, 0:1], op=ALU.mult)
            nc.vector.tensor_tensor(sq[:], sq[:], ag[:, 1:2], op=ALU.add)
            nc.vector.tensor_scalar(bnpack[:, 2 * mc + 1:2 * mc + 2], sq[:],
                                    float(L), None, op0=ALU.mult)
        if HOST_BN:
            # ship local sums + unnormalized activations; host finishes BN
            nc.sync.dma_start(d["stats"][:], bnpack[:])
            for cc in range(2):
                for hh in range(2):
                    sl = slice(2048 * hh, 2048 * (hh + 1))
                    nc.sync.dma_start(d["y"][128 * cc:128 * (cc + 1), sl],
                                      out_cm[cc][:, sl])
        else:
            cin = dram.tile([128, 4], F32, name="cin")
            cout = dram.tile([128, 4], F32, name="cout")
            nc.sync.dma_start(cin[:], bnpack[:])
            nc.gpsimd.collective_compute(
                "AllReduce", ALU.add, replica_groups=[list(range(n_cores))],
                ins=[cin.opt()], outs=[cout.opt()])
            gs = small.tile([128, 4], F32, tag="gs", name="gs")
            nc.sync.dma_start(gs[:], cout[:])

            NTOT = float(n_cores * L)
            scale = small.tile([128, 2], F32, tag="scale", name="scale")
            shift = small.tile([128, 2], F32, tag="shift", name="shift")
            mean = small.tile([128, 2], F32, tag="mean", name="mean")
            var = small.tile([128, 2], F32, tag="var", name="var")
            for cc in range(2):
                nc.vector.tensor_scalar(mean[:, cc:cc + 1], gs[:, 2 * cc:2 * cc + 1],
                                        1.0 / NTOT, None, op0=ALU.mult)
                nc.vector.tensor_scalar(var[:, cc:cc + 1], gs[:, 2 * cc + 1:2 * cc + 2],
                                        1.0 / NTOT, None, op0=ALU.mult)
            msq = small.tile([128, 2], F32, tag="msq", name="msq")
            nc.vector.tensor_tensor(msq[:], mean[:], mean[:], op=ALU.mult)
            nc.vector.tensor_tensor(var[:], var[:], msq[:], op=ALU.subtract)
            rs = small.tile([128, 2], F32, tag="rs", name="rs")
            nc.vector.tensor_scalar(var[:], var[:], float(EPS), None, op0=ALU.add)
            nc.scalar.activation(rs[:], var[:], ACT.Sqrt)
            nc.vector.reciprocal(rs[:], rs[:])
            nc.vector.tensor_tensor(scale[:], gamma2[:], rs[:], op=ALU.mult)
            nc.vector.tensor_tensor(shift[:], mean[:], scale[:], op=ALU.mult)
            nc.vector.tensor_tensor(shift[:], beta2[:], shift[:], op=ALU.subtract)

            for cc in range(2):
                for hh in range(2):
                    sl = slice(2048 * hh, 2048 * (hh + 1))
                    nc.vector.tensor_scalar(out_cm[cc][:, sl], out_cm[cc][:, sl],
                                            scale[:, cc:cc + 1], shift[:, cc:cc + 1],
                                            op0=ALU.mult, op1=ALU.add)
                    nc.sync.dma_start(d["y"][128 * cc:128 * (cc + 1), sl],
                                      out_cm[cc][:, sl])


_CACHE = {}


def _get_program(n_cores=N_CORES, debug=False):
    key = (n_cores, debug)
    if key not in _CACHE:
        nc = bacc.Bacc("TRN2", target_bir_lowering=False, debug=False,
                       num_devices=n_cores)
        build(nc, n_cores, debug)
        nc.compile()
        _CACHE[key] = nc
    return _CACHE[key]


def make_in_map(inputs, b):
    consts = host_consts()
    f16 = np.float16
    # host-side grid permutation: xf16[u*64+v, c] = x[v, u, c]
    xf16 = np.ascontiguousarray(
        np.asarray(inputs["x"][b]).transpose(1, 0, 2).reshape(L, C)).astype(f16)
    wv = np.ascontiguousarray(
        np.asarray(inputs["Wv"], np.float32).reshape(2, 128, C)
        .transpose(1, 0, 2).reshape(128, 2 * C)).astype(f16)
    wa = np.ascontiguousarray(
        np.asarray(inputs["Wa"], np.float32).reshape(2, 128, 81)
        .transpose(1, 0, 2).reshape(128, 2 * 81)).astype(f16)
    wfu = np.ascontiguousarray(
        np.asarray(inputs["Wfu"], np.float32).reshape(4, 128, 2, 128)
        .transpose(1, 0, 2, 3).reshape(128, 4 * 2 * 128)).astype(f16)
    return {
        "xf16": xf16,
        "wv": wv, "wa": wa, "wfu": wfu,
        "bvrow": np.asarray(inputs["bv"], np.float32).reshape(1, C).astype(f16),
        "barow": np.ascontiguousarray(
            np.asarray(inputs["ba"], np.float32).reshape(81, 1)),
        "bfurow": np.asarray(inputs["bfu"], np.float32).reshape(1, C).astype(f16),
        "gamma2": np.ascontiguousarray(
            np.asarray(inputs["gamma"], np.float32).reshape(2, 128).T),
        "beta2": np.ascontiguousarray(
            np.asarray(inputs["beta"], np.float32).reshape(2, 128).T),
        **consts,
    }


def gather_full(results, inputs):
    # y[c, u*64+v] -> out[u, v, c], cast fp16 -> f32 on host; with HOST_BN the
    # global batch-norm (exact, all 8 cores' stats) is applied here.
    ys = np.stack([np.asarray(results[b]["y"], dtype=np.float32)
                   for b in range(B)])                      # [B, C, L]
    if HOST_BN:
        st = np.stack([np.asarray(results[b]["stats"], dtype=np.float64)
                       for b in range(B)])                  # [B, 128, 4]
        st = st.sum(axis=0)
        s_pack = st.reshape(128, 2, 2)                      # [p, cc, (sum, sumsq)]
        cnt = float(B * L)
        mean = (s_pack[:, :, 0].T.reshape(C) / cnt)         # [C] (cc-major)
        ex2 = (s_pack[:, :, 1].T.reshape(C) / cnt)
        var = ex2 - mean * mean
        gamma = np.asarray(inputs["gamma"], np.float64)
        beta = np.asarray(inputs["beta"], np.float64)
        scale = gamma / np.sqrt(var + EPS)
        shift = beta - mean * scale
        ys = ys * scale[None, :, None] + shift[None, :, None]
    out = ys.reshape(B, C, H, W).transpose(0, 2, 3, 1)
    return np.ascontiguousarray(out, dtype=np.float32)


def kernel(**inputs):
    nc = _get_program()
    in_maps = [make_in_map(inputs, b) for b in range(B)]
    res = run_bass_kernel_spmd(nc, in_maps, list(range(N_CORES)))
    return gather_full(res.results, inputs)
